# revision 38
# baseline (speedup 1.0000x reference)
"""Self-contained Trainium2 Bass kernel for a Transformer encoder layer.

Reference computation (fp32):
    q,k,v = x@wq, x@wk, x@wv          (per-head split, DK=64)
    attn  = softmax(q@k^T/sqrt(DK) + mask_bias) @ v
    x1    = LN(x + attn@wo) * g1 + be1
    out   = LN(x1 + relu(x1@w1 + b1)@w2 + b2) * g2 + be2

Sharding: pure data-parallel over (batch, seq). 8 cores; core c owns batch
c//4 and a 1024-row query shard (c%4). K/V projections for the full batch
are computed redundantly on each core (no collectives).

Key implementation choices (v2):
  - All matmul operands bf16 (weights host-cast): enables Fast Weight Load
    (4x faster LDWEIGHTS vs fp32r) at identical PE streaming rate; psum
    accumulation stays fp32.
  - K^T / V / Q^T stay SBUF-resident (no DRAM spill, no phase barrier).
  - softmax exp runs mostly on the Vector engine via the Schraudolph bit
    trick (int16(s*A+B) bitcast bf16, ~+-3% relative, largely cancelling
    between numerator and rowsum), a fraction on ACT (true Exp) for load
    balance. Mask bias folds into the trick's additive constant; fully
    masked scores saturate the int16 convert to -32768 -> bf16 -0.0.
  - softmax rowsum rides the ctx matmul as a ones column of V (M=65);
    normalization = ACT row copy + gpsimd partition_broadcast +
    reciprocal_approx_fast + one DVE multiply, all on-chip.
  - 1-deep software pipelines (scores(kt+1) ahead of ctx(kt), transposes
    (sl+1) ahead of proj MMs(sl)) keep the PE from stalling on exp/copies.
"""

import os
import sys

import numpy as np

if os.path.isdir("/opt/trn_rl_repo") and "/opt/trn_rl_repo" not in sys.path:
    sys.path.insert(0, "/opt/trn_rl_repo")

import ml_dtypes

import concourse.bacc as bacc
import concourse.bass as bass
import concourse.tile as tile
from concourse import mybir
from concourse.bass_utils import run_bass_kernel_spmd
from concourse.masks import make_identity

B, S, D, H, DK = 2, 4096, 512, 8, 64
DFF = 2048
EPS = 1e-5
N_CORES = 8
SHARD = S // 4  # 1024 query rows per core
F32 = mybir.dt.float32
BF16 = mybir.dt.bfloat16
I16 = mybir.dt.int16
I8 = mybir.dt.int8
FP8 = mybir.dt.float8e4
AF = mybir.ActivationFunctionType
ALU = mybir.AluOpType
PM_DR = mybir.MatmulPerfMode.DoubleRow

NDC_H = D // 128         # host-side chunk count (= NDC)
NJC = D // 256           # ko-pair chunks for fp8 DoubleRow projections
NSLICE = S // 512        # 8 column slices of x^T
NQSL = SHARD // 512      # 2 slices for the Q shard
NPAIR = H // 2           # 4 head pairs
NKT = S // 128           # 32 key tiles
NQT = SHARD // 128       # 8 query tiles in the shard
NDC = D // 128           # 4 contraction chunks of D
NFC = DFF // 128         # 16 chunks of DFF

# Schraudolph fast-exp constants, fp8e4m3 output via int8 bit trick:
# p~ = bitcast_fp8(int8(max(s*EXA + bias, 0))).  K8 folds a 2^-4 scale
# into every p (softmax is scale-invariant; the rowsum rides along): the
# ACT true-exp path then peaks at e^8.03*2^-4 = 193, under the ~240
# ceiling where the hardware ACT->fp8 convert overflows to inf, and the
# DVE int8 trick peaks at t=117, well under the 127=NaN encoding
# regardless of convert rounding mode (no +0.5: the int8 convert was
# observed to round, unlike the truncating int16 convert).
LOG2E = 1.4426950408889634
EXC = 0.0303
K8 = -4.0
EXA = 8.0 * LOG2E
EXB = (7.0 - EXC) * 8.0 + 8.0 * K8
NKP = NKT // 2           # key-tile pairs (fp8 DoubleRow contracts 2 tiles)


def _build_program(apply_affine1, apply_affine2, apply_b2):
    nc = bacc.Bacc("TRN2", target_bir_lowering=False, debug=False,
                   num_devices=N_CORES)

    # host-pre-transposed activations in the fp8 DoubleRow layout
    # [p, j, k, s] = x[s, 256j+128k+p] -- serves as DR lhsT (V) and DR rhs
    # (K/Q): both sides carry the ko dim ahead of the streamed dim.
    xT8 = nc.declare_dram_parameter("xT8", [128, NJC, 2, S], FP8, isOutput=False)
    xqT8 = nc.declare_dram_parameter("xqT8", [128, NJC, 2, SHARD], FP8, isOutput=False)
    xq = nc.declare_dram_parameter("xq", [SHARD, D], F32, isOutput=False)
    # mask biases, host-packed [p, t] = bias[t*128+p] so the DMA is contiguous
    mbt = nc.declare_dram_parameter("mbt", [128, NKT], F32, isOutput=False)
    mbr = nc.declare_dram_parameter("mbr", [128, NKT], F32, isOutput=False)
    # q/k/v weights: fp8, host-scaled x8 (good e4m3 range); corrective
    # scales ride the free psum->sbuf copy affine (kTt: /8; qT: EXA/(8*8))
    wq8 = nc.declare_dram_parameter("wq8", [128, NJC, 2, D], FP8, isOutput=False)
    wk8 = nc.declare_dram_parameter("wk8", [128, NJC, 2, D], FP8, isOutput=False)
    wv8 = nc.declare_dram_parameter("wv8", [128, NJC, 2, D], FP8, isOutput=False)
    wo = nc.declare_dram_parameter("wo", [D, D], BF16, isOutput=False)
    w1 = nc.declare_dram_parameter("w1", [D, DFF], BF16, isOutput=False)
    b1 = nc.declare_dram_parameter("b1", [DFF], F32, isOutput=False)
    w2 = nc.declare_dram_parameter("w2", [DFF, D], BF16, isOutput=False)
    b2 = nc.declare_dram_parameter("b2", [D], F32, isOutput=False)
    g1 = nc.declare_dram_parameter("g1", [D], F32, isOutput=False)
    be1 = nc.declare_dram_parameter("be1", [D], F32, isOutput=False)
    g2 = nc.declare_dram_parameter("g2", [D], F32, isOutput=False)
    be2 = nc.declare_dram_parameter("be2", [D], F32, isOutput=False)
    out = nc.declare_dram_parameter("out", [SHARD, D], F32, isOutput=True)

    def bcast_ap(vec, parts):
        a = vec if isinstance(vec, bass.AP) else vec.ap()
        ap_dims = [list(d) for d in a.ap]
        if len(ap_dims) > 1 and ap_dims[0][1] == 1:
            ap_dims = ap_dims[1:]
        return bass.AP(tensor=a.tensor, offset=a.offset,
                       ap=[[0, parts]] + ap_dims)

    import contextlib
    with tile.TileContext(nc, pool_alloc_mode="queue") as tc, \
         contextlib.ExitStack() as ctx:
        consts = ctx.enter_context(tc.tile_pool(name="consts", bufs=1))
        ident = consts.tile([128, 128], F32)
        make_identity(nc, ident)
        mbt_t = consts.tile([128, NKT], F32)
        mbr_t = consts.tile([128, NKT], F32)
        epst = consts.tile([128, 1], F32)
        nc.vector.memset(epst, EPS)

        # late-phase weights: tiles allocated here (pool stack order), DMAs
        # emitted inside phase 1 after the critical slice-0 loads
        wlate = ctx.enter_context(tc.tile_pool(name="wlate", bufs=1))
        wo_sb = wlate.tile([64, H, D], BF16)
        w1_sb = wlate.tile([128, NDC, DFF], BF16)
        w2_sb = wlate.tile([128, NFC, D], BF16)
        b1_sb = wlate.tile([128, NFC], F32)
        b2b = g1b = be1b = g2b = be2b = None
        if apply_b2:
            b2b = wlate.tile([128, D], F32)
        if apply_affine1:
            g1b = wlate.tile([128, D], F32)
            be1b = wlate.tile([128, D], F32)
        if apply_affine2:
            g2b = wlate.tile([128, D], F32)
            be2b = wlate.tile([128, D], F32)

        def emit_wlate_dmas():
            nc.sync.dma_start(out=wo_sb, in_=wo.ap().rearrange("(h p) n -> p h n", p=64))
            nc.sync.dma_start(out=w1_sb, in_=w1.ap().rearrange("(c p) n -> p c n", p=128))
            nc.sync.dma_start(out=w2_sb, in_=w2.ap().rearrange("(f p) n -> p f n", p=128))
            nc.sync.dma_start(out=b1_sb, in_=b1.ap().rearrange("(f p) -> p f", p=128))
            if apply_b2:
                nc.sync.dma_start(out=b2b, in_=bcast_ap(b2, 128))
            if apply_affine1:
                nc.sync.dma_start(out=g1b, in_=bcast_ap(g1, 128))
                nc.sync.dma_start(out=be1b, in_=bcast_ap(be1, 128))
            if apply_affine2:
                nc.sync.dma_start(out=g2b, in_=bcast_ap(g2, 128))
                nc.sync.dma_start(out=be2b, in_=bcast_ap(be2, 128))

        # normalized ctx^T per head — written in phase 2, read in phase 3;
        # allocated before the attn pool so pools release in stack order
        ln1 = ctx.enter_context(tc.tile_pool(name="ln1", bufs=1))
        cn = ln1.tile([64, H, SHARD], BF16)

        # ---- residents for phases 1-2 ---------------------------------
        es_attn = contextlib.ExitStack()
        attn_pool = es_attn.enter_context(tc.tile_pool(name="attn", bufs=1))
        kTt = attn_pool.tile([128, NPAIR, S], BF16)          # pair-packed K^T
        # V + ones col, fp8, kt-pair-interleaved for DoubleRow ctx matmuls;
        # +2 pad makes the DoubleRow Ko stride (H*(DK+2) = 528B) 16-aligned
        v1 = attn_pool.tile([128, NKP, 2, H, DK + 2], FP8)
        nc.vector.memset(v1[:, :, :, :, DK:DK + 1], 1.0)
        qT = attn_pool.tile([128, NPAIR, SHARD], BF16)       # pair-packed Q^T

        # ---- phase 1: fp8 DoubleRow projections -----------------------
        SQ8 = float(EXA / (8.0 * np.sqrt(DK)))
        with tc.tile_pool(name="pw", bufs=1) as pw, \
             tc.tile_pool(name="p1p", bufs=3, space="PSUM") as p1p:
            wk_sb = pw.tile([128, NJC, 2, D], FP8)
            nc.sync.dma_start(out=wk_sb, in_=wk8.ap())
            wv_sb = pw.tile([128, NJC, 2, D], FP8)
            nc.sync.dma_start(out=wv_sb, in_=wv8.ap())
            xT_sb = pw.tile([128, NJC, 2, S], FP8)

            def load_slice(sl):
                s0 = sl * 512
                nc.sync.dma_start(out=xT_sb[:, :, :, s0:s0 + 512],
                                  in_=xT8.ap()[:, :, :, s0:s0 + 512])

            for sl in range(2):
                load_slice(sl)
            wq_sb = pw.tile([128, NJC, 2, D], FP8)
            nc.sync.dma_start(out=wq_sb, in_=wq8.ap())
            xqTs = pw.tile([128, NJC, 2, SHARD], FP8)
            nc.sync.dma_start(out=xqTs, in_=xqT8.ap())
            nc.sync.dma_start(out=mbt_t, in_=mbt.ap())
            nc.sync.dma_start(out=mbr_t, in_=mbr.ap())
            for sl in range(2, NSLICE):
                load_slice(sl)
            emit_wlate_dmas()

            def emit_proj(sl):
                """K^T and V matmuls for full-batch slice sl (fp8 DR)."""
                s0 = sl * 512
                for pr in range(NPAIR):
                    kp = p1p.tile([128, 512], F32, tag="kpsum")
                    for j in range(NJC):
                        nc.tensor.matmul(kp, wk_sb[:, j, :, pr * 128:(pr + 1) * 128],
                                         xT_sb[:, j, :, s0:s0 + 512],
                                         start=(j == 0), stop=(j == NJC - 1),
                                         perf_mode=PM_DR)
                    nc.scalar.activation(out=kTt[:, pr, s0:s0 + 512], in_=kp,
                                         func=AF.Copy, bias=0.0, scale=0.125)
                for m in range(4):
                    vp = p1p.tile([128, 512], F32, tag="kpsum")
                    for j in range(NJC):
                        nc.tensor.matmul(vp, xT_sb[:, j, :, s0 + m * 128:s0 + (m + 1) * 128],
                                         wv_sb[:, j, :, :],
                                         start=(j == 0), stop=(j == NJC - 1),
                                         perf_mode=PM_DR)
                    kt_ = sl * 4 + m
                    # vp holds 8*v; the 1/8 is folded into wo host-side
                    nc.vector.tensor_copy(
                        out=v1[:, kt_ // 2, kt_ % 2, :, 0:DK], in_=vp)

            def emit_q(qsl):
                for pr in range(NPAIR):
                    qp = p1p.tile([128, 512], F32, tag="kpsum")
                    for j in range(NJC):
                        nc.tensor.matmul(qp, wq_sb[:, j, :, pr * 128:(pr + 1) * 128],
                                         xqTs[:, j, :, qsl * 512:(qsl + 1) * 512],
                                         start=(j == 0), stop=(j == NJC - 1),
                                         perf_mode=PM_DR)
                    nc.scalar.activation(out=qT[:, pr, qsl * 512:(qsl + 1) * 512],
                                         in_=qp, func=AF.Copy, bias=0.0, scale=SQ8)

            for sl in range(NSLICE):
                emit_proj(sl)
            for qsl in range(NQSL):
                emit_q(qsl)

        # ---- phase 2: attention ---------------------------------------
        with tc.tile_pool(name="ppool", bufs=6) as ppool, \
             tc.tile_pool(name="rpool", bufs=3) as rpool, \
             tc.tile_pool(name="spsum", bufs=2, space="PSUM") as spsum, \
             tc.tile_pool(name="cpsum", bufs=2, space="PSUM") as cpsum:
            for pr in range(NPAIR):
                cA = cpsum.tile([DK + 1, SHARD], F32, tag="ctx")
                cB = cpsum.tile([DK + 1, SHARD], F32, tag="ctx")
                # scores/exp stream per kt; fp8 DoubleRow ctx per kt-PAIR,
                # emitted 1.5 pairs behind so the ~1.2us exp never stalls
                # the PE.  pT tiles hold both kts of a pair interleaved.
                cur = {}
                hist = {}
                for kt in range(NKT + 2):
                    if kt < NKT:
                        j = kt % 2
                        for qh in range(NQSL):
                            sp = spsum.tile([128, SHARD], F32, tag="scores")
                            for hh in (0, 1):
                                lo, hi = hh * 64, hh * 64 + 64
                                nc.tensor.matmul(
                                    sp[:, hh * 512:(hh + 1) * 512],
                                    kTt[lo:hi, pr, kt * 128:(kt + 1) * 128],
                                    qT[lo:hi, pr, qh * 512:(qh + 1) * 512],
                                    start=True, stop=True)
                            if j == 0:
                                pTnew = ppool.tile([128, 2, SHARD], I8,
                                                   tag=f"pT{qh}")
                                cur[qh] = pTnew
                            pT = cur[qh]
                            # scores arrive pre-scaled by EXA (folded into
                            # wq host-side). Strict per-kt alternation keeps
                            # both engines under the PE's per-kt budget.
            # exp runs as two half-width ops IN PARALLEL: the hh0/hh1
                            # column halves live in different psum banks, so
                            # ACT (true exp, even head) and DVE (Schraudolph
                            # trick, odd head) read them concurrently.  This
                            # halves the scores-ring round-trip (the phase's
                            # critical cycle) vs one full-width exp.
                            nc.scalar.activation(
                                pT[:, j, 0:512].bitcast(FP8), sp[:, 0:512],
                                AF.Exp, bias=mbr_t[:, kt:kt + 1],
                                scale=1.0 / EXA)
                            nc.vector.tensor_scalar(
                                out=pT[:, j, 512:1024], in0=sp[:, 512:1024],
                                scalar1=mbt_t[:, kt:kt + 1], scalar2=0.0,
                                op0=ALU.add, op1=ALU.max)
                        if j == 1:
                            hist[kt // 2] = (cur[0], cur[1])
                    if kt % 2 == 1 and kt >= 3:
                        ktp = (kt - 3) // 2
                        p0, p1 = hist.pop(ktp)
                        p08, p18 = p0.bitcast(FP8), p1.bitcast(FP8)
                        for hh, cps in ((0, cA), (1, cB)):
                            h = 2 * pr + hh
                            va = v1[:, ktp, 0, h, 0:DK + 1]
                            lhsT = bass.AP(
                                tensor=va.tensor, offset=va.offset,
                                ap=[list(va.ap[0]), [H * (DK + 2), 2],
                                    [1, DK + 1]])
                            for qh, p8 in ((0, p08), (1, p18)):
                                nc.tensor.matmul(
                                    cps[:, qh * 512:(qh + 1) * 512],
                                    lhsT,
                                    p8[:, :, hh * 512:(hh + 1) * 512],
                                    start=(ktp == 0), stop=(ktp == NKP - 1),
                                    perf_mode=PM_DR)
                # normalize: stage ctx to SBUF first (frees the psum bank
                # immediately -- the next pair's ctx accumulators and
                # phase 3's wo accumulators reuse those banks), approx-
                # recip the 1-partition rowsum, gpsimd-broadcast the
                # reciprocal, then multiply: pairs 0-2 on gpsimd, last
                # pair on DVE (whose exp work is done by then).
                last = (pr == NPAIR - 1)
                for hh, cps in ((0, cA), (1, cB)):
                    h = 2 * pr + hh
                    # NOTE: the rowsum lives at psum partition 64; only the
                    # ACT copy can shift it to partition 0 (DVE/gpsimd lanes
                    # are partition-hardwired -- a DVE op with in/out at
                    # different base partitions silently breaks on HW).
                    rsr = rpool.tile([1, SHARD], F32, tag="rsr")
                    nc.scalar.copy(out=rsr, in_=cps[DK:DK + 1, :])
                    stg = rpool.tile([DK, SHARD], F32, tag="stg")
                    if last:
                        # DVE is exp-idle by now; evacuating on DVE runs in
                        # parallel with the ACT rsr copies, freeing the ctx
                        # psum banks ~2us sooner for phase 3's wo pools
                        nc.vector.tensor_copy(out=stg, in_=cps[0:DK, :])
                    else:
                        nc.scalar.copy(out=stg, in_=cps[0:DK, :])
                    rb = rpool.tile([64, SHARD], F32, tag="rb")
                    nc.gpsimd.partition_broadcast(rb, rsr)
                    nc.vector.reciprocal_approx_fast(out=rb, in_=rb)
                    if last:
                        nc.vector.tensor_mul(out=cn[:, h, :],
                                             in0=stg[0:DK, :], in1=rb)
                    else:
                        nc.gpsimd.tensor_mul(out=cn[:, h, :],
                                             in0=stg[0:DK, :], in1=rb)
        es_attn.close()  # free kTt + v1 + qT

        # ---- phases 3-5: wo+LN1, x1^T, FFN1, FFN2+LN2 -----------------
        # wo runs in two waves over 8 psum banks: heads 0-5 for every row
        # tile first (only needs pairs 0-2), then heads 6-7 + the LN chain.
        # The first wave (~11us of PE work) hides the last pair's softmax
        # normalization latency, so the PE never idles at the boundary and
        # HAM stays warm into the FFN.
        late2 = ctx.enter_context(tc.tile_pool(name="late2", bufs=1))
        x1T = late2.tile([128, NDC, SHARD], BF16)
        x1keep = late2.tile([128, NQT, D], F32)
        h1T = late2.tile([128, NFC, SHARD], BF16)
        with tc.tile_pool(name="p3s", bufs=1) as p3s, \
             tc.tile_pool(name="p3t", bufs=4) as p3t, \
             tc.tile_pool(name="p3st", bufs=4) as p3st, \
             tc.tile_pool(name="p5s", bufs=3) as p5s, \
             tc.tile_pool(name="p5st", bufs=4) as p5st, \
             contextlib.ExitStack() as es_ffn:
            # two 4-bank pools: pool A reuses the scores psum banks (freed
            # at the last exp) so wo can start before the ctx banks drain
            es_ap = contextlib.ExitStack()
            p3apA = es_ap.enter_context(
                tc.tile_pool(name="p3apA", bufs=1, space="PSUM"))
            p3apB = es_ap.enter_context(
                tc.tile_pool(name="p3apB", bufs=1, space="PSUM"))
            aps = {}
            xts = {}

            def emit_wo_a(m):
                pool = p3apA if m < 4 else p3apB
                ap_ = pool.tile([128, D], F32, tag=f"ap{m}")
                xt = p3s.tile([128, D], F32, tag=f"xres{m}")
                nc.sync.dma_start(out=xt, in_=xq[m * 128:(m + 1) * 128, :])
                aps[m], xts[m] = ap_, xt
                for h in range(6):
                    nc.tensor.matmul(ap_, cn[:, h, m * 128:(m + 1) * 128],
                                     wo_sb[:, h, :], start=(h == 0), stop=False)

            def emit_wo_b(m):
                ap_, xt = aps[m], xts[m]
                for h in (6, 7):
                    nc.tensor.matmul(ap_, cn[:, h, m * 128:(m + 1) * 128],
                                     wo_sb[:, h, :], start=False,
                                     stop=(h == H - 1))
                t = p3t.tile([128, D], F32, tag="tres")
                nc.vector.tensor_add(out=t, in0=ap_, in1=xt)
                stats = p3st.tile([128, 6], F32, tag="stats")
                nc.vector.bn_stats(out=stats, in_=t)
                mv = p3st.tile([128, 2], F32, tag="mv")
                nc.vector.bn_aggr(out=mv, in_=stats)
                sd = p3st.tile([128, 1], F32, tag="sd")
                nc.scalar.activation(out=sd, in_=mv[:, 1:2], func=AF.Sqrt,
                                     bias=epst, scale=1.0)
                rs = p3st.tile([128, 1], F32, tag="rs")
                nc.vector.reciprocal_approx_fast(out=rs, in_=sd)
                x1m = x1keep[:, m, :]
                nc.vector.tensor_scalar(out=x1m, in0=t, scalar1=mv[:, 0:1],
                                        scalar2=rs, op0=ALU.subtract, op1=ALU.mult)
                if apply_affine1:
                    nc.vector.tensor_mul(out=x1m, in0=x1m, in1=g1b)
                    nc.vector.tensor_add(out=x1m, in0=x1m, in1=be1b)

            def emit_x1t(m):
                # transpose into the psum bank freed by this m's wo
                # accumulator (same pool tag -> same bank, dep-ordered)
                x1m = x1keep[:, m, :]
                pool = p3apA if m < 4 else p3apB
                tp = pool.tile([128, 512], F32, tag=f"ap{m}")
                for c in range(NDC):
                    nc.tensor.transpose(tp[:, c * 128:(c + 1) * 128],
                                        x1m[:, c * 128:(c + 1) * 128], ident)
                nc.scalar.copy(out=x1T[:, :, m * 128:(m + 1) * 128], in_=tp)

            def emit_ffn1(qh, p4p):
                for f in range(NFC):
                    hp = p4p.tile([128, 512], F32, tag="hpsum")
                    for c in range(NDC):
                        nc.tensor.matmul(hp,
                                         w1_sb[:, c, f * 128:(f + 1) * 128],
                                         x1T[:, c, qh * 512:(qh + 1) * 512],
                                         start=(c == 0), stop=(c == NDC - 1))
                    nc.scalar.activation(out=h1T[:, f, qh * 512:(qh + 1) * 512],
                                         in_=hp, func=AF.Relu,
                                         bias=b1_sb[:, f:f + 1], scale=1.0)

            def emit_ffn2(m, p5p):
                fp = p5p.tile([128, D], F32, tag="fpsum")
                for f in range(NFC):
                    nc.tensor.matmul(fp, h1T[:, f, m * 128:(m + 1) * 128],
                                     w2_sb[:, f, :], start=(f == 0),
                                     stop=(f == NFC - 1))
                t2 = p5s.tile([128, D], F32, tag="t2")
                nc.vector.tensor_add(out=t2, in0=fp, in1=x1keep[:, m, :])
                if apply_b2:
                    nc.vector.tensor_add(out=t2, in0=t2, in1=b2b)
                stats = p5st.tile([128, 6], F32, tag="stats5")
                nc.vector.bn_stats(out=stats, in_=t2)
                mv = p5st.tile([128, 2], F32, tag="mv5")
                nc.vector.bn_aggr(out=mv, in_=stats)
                sd = p5st.tile([128, 1], F32, tag="sd5")
                nc.scalar.activation(out=sd, in_=mv[:, 1:2], func=AF.Sqrt,
                                     bias=epst, scale=1.0)
                rs = p5st.tile([128, 1], F32, tag="rs5")
                nc.vector.reciprocal_approx_fast(out=rs, in_=sd)
                o = p5s.tile([128, D], F32, tag="otile")
                nc.vector.tensor_scalar(out=o, in0=t2, scalar1=mv[:, 0:1],
                                        scalar2=rs, op0=ALU.subtract, op1=ALU.mult)
                if apply_affine2:
                    nc.vector.tensor_mul(out=o, in0=o, in1=g2b)
                    nc.vector.tensor_add(out=o, in0=o, in1=be2b)
                nc.sync.dma_start(out=out[m * 128:(m + 1) * 128, :], in_=o)

            for m in range(NQT):
                emit_wo_a(m)
            # interleave the h6/h7 tails with the x1 transposes: tp(m)
            # waits on LN(m) (DVE), which trails wo_b(m) by ~2.5us, so
            # keep 3 wo_b's of PE work in between.
            for m in range(3):
                emit_wo_b(m)
            for m in range(NQT):
                if m + 3 < NQT:
                    emit_wo_b(m + 3)
                emit_x1t(m)
            es_ap.close()  # free the 8 wo psum banks
            p4p = es_ffn.enter_context(
                tc.tile_pool(name="p4p", bufs=3, space="PSUM"))
            emit_ffn1(0, p4p)
            p5p = es_ffn.enter_context(
                tc.tile_pool(name="p5p", bufs=2, space="PSUM"))
            for m in range(4):
                emit_ffn2(m, p5p)
            emit_ffn1(1, p4p)
            for m in range(4, NQT):
                emit_ffn2(m, p5p)

    nc.compile()
    return nc


_PROG_CACHE = {}


def _get_program(key):
    if key not in _PROG_CACHE:
        _PROG_CACHE[key] = _build_program(*key)
    return _PROG_CACHE[key]


def _make_in_maps(x, mask, wq, wk, wv, wo, w1, b1, w2, b2, g1, be1, g2, be2):
    f = np.float32
    bf = ml_dtypes.bfloat16
    f8 = ml_dtypes.float8_e4m3fn
    # q/k/v weights x8 into fp8 (sigma ~0.35, comfortably normal-range);
    # the attention scale + fast-exp EXA ride the qT copy scale on-device,
    # and v arrives x8 so wo absorbs a 1/8.
    mraw0 = np.where(np.asarray(mask)[:, 0, 0, :] == 0, f(-1e9), f(0.0))
    mtrick = mraw0 * f(EXA) + f(EXB)
    mraw = mraw0 + f(K8 * np.log(2.0))  # ACT path: fold the 2^K8 shift

    def wlay(w):    # [D, D] -> DR layout [128, NJC, 2, D]
        a = (np.asarray(w, f) * 8).astype(f8)
        return np.ascontiguousarray(a.reshape(NJC, 2, 128, D).transpose(2, 0, 1, 3))

    shared = dict(
        wq8=wlay(wq), wk8=wlay(wk), wv8=wlay(wv),
        wo=(np.asarray(wo, f) / 8).astype(bf), w1=np.asarray(w1, f).astype(bf),
        b1=np.ascontiguousarray(b1, f), w2=np.asarray(w2, f).astype(bf),
        b2=np.ascontiguousarray(b2, f), g1=np.ascontiguousarray(g1, f),
        be1=np.ascontiguousarray(be1, f), g2=np.ascontiguousarray(g2, f),
        be2=np.ascontiguousarray(be2, f),
    )
    # pre-transposed fp8 activations per batch in the DR layout
    xt_list = []
    for b in range(B):
        x8 = np.asarray(x[b], f).T.astype(f8)          # [D, S]
        xt_list.append(np.ascontiguousarray(
            x8.reshape(NJC, 2, 128, S).transpose(2, 0, 1, 3)))
    in_maps = []
    for c in range(N_CORES):
        b, sh = c // 4, c % 4
        m = dict(shared)
        m["xT8"] = xt_list[b]
        m["xqT8"] = np.ascontiguousarray(
            xt_list[b][:, :, :, sh * SHARD:(sh + 1) * SHARD])
        m["xq"] = np.ascontiguousarray(x[b, sh * SHARD:(sh + 1) * SHARD], f)
        m["mbr"] = np.ascontiguousarray(mraw[b].reshape(NKT, 128).T)
        m["mbt"] = np.ascontiguousarray(mtrick[b].reshape(NKT, 128).T)
        in_maps.append(m)
    return in_maps


def kernel(x, mask, wq, wk, wv, wo, w1, b1, w2, b2, g1, be1, g2, be2,
           _trace=False, _tmpdir=None):
    key = (
        not (np.all(np.asarray(g1) == 1.0) and np.all(np.asarray(be1) == 0.0)),
        not (np.all(np.asarray(g2) == 1.0) and np.all(np.asarray(be2) == 0.0)),
        not np.all(np.asarray(b2) == 0.0),
    )
    nc = _get_program(key)
    in_maps = _make_in_maps(x, mask, wq, wk, wv, wo, w1, b1, w2, b2,
                            g1, be1, g2, be2)
    res = None
    for attempt in range(3):
        try:
            res = run_bass_kernel_spmd(nc, in_maps, list(range(N_CORES)),
                                       trace=_trace, tmpdir=_tmpdir)
            break
        except Exception:
            if attempt == 2:
                raise
            import time as _time
            _time.sleep(2.0)
    outs = [res.results[c]["out"] for c in range(N_CORES)]
    full = np.empty((B, S, D), np.float32)
    for c in range(N_CORES):
        b, sh = c // 4, c % 4
        full[b, sh * SHARD:(sh + 1) * SHARD] = outs[c]
    kernel._last_results = res
    return full



# revision 41
# speedup vs baseline: 1.1894x; 1.1894x over previous
"""Self-contained Trainium2 Bass kernel for a Transformer encoder layer.

Reference computation (fp32):
    q,k,v = x@wq, x@wk, x@wv          (per-head split, DK=64)
    attn  = softmax(q@k^T/sqrt(DK) + mask_bias) @ v
    x1    = LN(x + attn@wo) * g1 + be1
    out   = LN(x1 + relu(x1@w1 + b1)@w2 + b2) * g2 + be2

Sharding: pure data-parallel over (batch, seq). 8 cores; core c owns batch
c//4 and a 1024-row query shard (c%4). K/V projections for the full batch
are computed redundantly on each core (no collectives).

Key implementation choices (v3):
  - x arrives host-pre-transposed in the fp8 DoubleRow layout, so phase 1
    is pure DR matmul streaming (no PE transposes, no DVE staging copies):
    q/k/v projections run as fp8e4m3 DoubleRow MMs (weights host-scaled x8
    for e4m3 range; corrective scales ride the psum->sbuf copy affine).
  - FFN / wo matmul operands bf16 (host-cast); psum accumulation fp32.
  - K^T / V / Q^T stay SBUF-resident (no DRAM spill, no phase barrier).
  - phase 3 opens all 8 wo psum accumulators across two 4-bank pools
    (pool A reuses the early-freed scores banks), emits wo heads 0-5 for
    every row tile before heads 6-7, so ~11us of PE work covers the last
    attention pair's softmax normalization and HAM never re-throttles at
    the phase boundary; x1^T transposes then reuse each tile's freed bank.
  - softmax exp runs mostly on the Vector engine via the Schraudolph bit
    trick (int16(s*A+B) bitcast bf16, ~+-3% relative, largely cancelling
    between numerator and rowsum), a fraction on ACT (true Exp) for load
    balance. Mask bias folds into the trick's additive constant; fully
    masked scores saturate the int16 convert to -32768 -> bf16 -0.0.
  - softmax rowsum rides the ctx matmul as a ones column of V (M=65);
    normalization = ACT row copy + gpsimd partition_broadcast +
    reciprocal_approx_fast + one DVE multiply, all on-chip.
  - 1-deep software pipelines (scores(kt+1) ahead of ctx(kt), transposes
    (sl+1) ahead of proj MMs(sl)) keep the PE from stalling on exp/copies.
"""

import os
import sys

import numpy as np

if os.path.isdir("/opt/trn_rl_repo") and "/opt/trn_rl_repo" not in sys.path:
    sys.path.insert(0, "/opt/trn_rl_repo")

import ml_dtypes

import concourse.bacc as bacc
import concourse.bass as bass
import concourse.tile as tile
from concourse import mybir
from concourse.bass_utils import run_bass_kernel_spmd
from concourse.masks import make_identity

B, S, D, H, DK = 2, 4096, 512, 8, 64
DFF = 2048
EPS = 1e-5
N_CORES = 8
SHARD = S // 4  # 1024 query rows per core
F32 = mybir.dt.float32
BF16 = mybir.dt.bfloat16
I16 = mybir.dt.int16
I8 = mybir.dt.int8
FP8 = mybir.dt.float8e4
AF = mybir.ActivationFunctionType
ALU = mybir.AluOpType
PM_DR = mybir.MatmulPerfMode.DoubleRow

NDC_H = D // 128         # host-side chunk count (= NDC)
NJC = D // 256           # ko-pair chunks for fp8 DoubleRow projections
NSLICE = S // 512        # 8 column slices of x^T
NQSL = SHARD // 512      # 2 slices for the Q shard
NPAIR = H // 2           # 4 head pairs
NKT = S // 128           # 32 key tiles
NQT = SHARD // 128       # 8 query tiles in the shard
NDC = D // 128           # 4 contraction chunks of D
NFC = DFF // 128         # 16 chunks of DFF

# Schraudolph fast-exp constants, fp8e4m3 output via int8 bit trick:
# p~ = bitcast_fp8(int8(max(s*EXA + bias, 0))).  K8 folds a 2^-4 scale
# into every p (softmax is scale-invariant; the rowsum rides along): the
# ACT true-exp path then peaks at e^8.03*2^-4 = 193, under the ~240
# ceiling where the hardware ACT->fp8 convert overflows to inf, and the
# DVE int8 trick peaks at t=117, well under the 127=NaN encoding
# regardless of convert rounding mode (no +0.5: the int8 convert was
# observed to round, unlike the truncating int16 convert).
LOG2E = 1.4426950408889634
EXC = 0.0303
K8 = -4.0
EXA = 8.0 * LOG2E
EXB = (7.0 - EXC) * 8.0 + 8.0 * K8
NKP = NKT // 2           # key-tile pairs (fp8 DoubleRow contracts 2 tiles)


def _build_program(apply_affine1, apply_affine2, apply_b2):
    nc = bacc.Bacc("TRN2", target_bir_lowering=False, debug=False,
                   num_devices=N_CORES)

    # host-pre-transposed activations in the fp8 DoubleRow layout
    # [p, j, k, s] = x[s, 256j+128k+p] -- serves as DR lhsT (V) and DR rhs
    # (K/Q): both sides carry the ko dim ahead of the streamed dim.
    xT8 = nc.declare_dram_parameter("xT8", [128, NJC, 2, S], FP8, isOutput=False)
    xqT8 = nc.declare_dram_parameter("xqT8", [128, NJC, 2, SHARD], FP8, isOutput=False)
    xq = nc.declare_dram_parameter("xq", [SHARD, D], F32, isOutput=False)
    # mask biases, host-packed [p, t] = bias[t*128+p] so the DMA is contiguous
    mbt = nc.declare_dram_parameter("mbt", [128, NKT], F32, isOutput=False)
    mbr = nc.declare_dram_parameter("mbr", [128, NKT], F32, isOutput=False)
    # q/k/v weights: fp8, host-scaled x8 (good e4m3 range); corrective
    # scales ride the free psum->sbuf copy affine (kTt: /8; qT: EXA/(8*8))
    wq8 = nc.declare_dram_parameter("wq8", [128, NJC, 2, D], FP8, isOutput=False)
    wk8 = nc.declare_dram_parameter("wk8", [128, NJC, 2, D], FP8, isOutput=False)
    wv8 = nc.declare_dram_parameter("wv8", [128, NJC, 2, D], FP8, isOutput=False)
    wo = nc.declare_dram_parameter("wo", [D, D], BF16, isOutput=False)
    w1 = nc.declare_dram_parameter("w1", [D, DFF], BF16, isOutput=False)
    b1 = nc.declare_dram_parameter("b1", [DFF], F32, isOutput=False)
    w2 = nc.declare_dram_parameter("w2", [DFF, D], BF16, isOutput=False)
    b2 = nc.declare_dram_parameter("b2", [D], F32, isOutput=False)
    g1 = nc.declare_dram_parameter("g1", [D], F32, isOutput=False)
    be1 = nc.declare_dram_parameter("be1", [D], F32, isOutput=False)
    g2 = nc.declare_dram_parameter("g2", [D], F32, isOutput=False)
    be2 = nc.declare_dram_parameter("be2", [D], F32, isOutput=False)
    out = nc.declare_dram_parameter("out", [SHARD, D], F32, isOutput=True)

    def bcast_ap(vec, parts):
        a = vec if isinstance(vec, bass.AP) else vec.ap()
        ap_dims = [list(d) for d in a.ap]
        if len(ap_dims) > 1 and ap_dims[0][1] == 1:
            ap_dims = ap_dims[1:]
        return bass.AP(tensor=a.tensor, offset=a.offset,
                       ap=[[0, parts]] + ap_dims)

    import contextlib
    with tile.TileContext(nc, pool_alloc_mode="queue") as tc, \
         contextlib.ExitStack() as ctx:
        consts = ctx.enter_context(tc.tile_pool(name="consts", bufs=1))
        ident = consts.tile([128, 128], F32)
        make_identity(nc, ident)
        mbt_t = consts.tile([128, NKT], F32)
        mbr_t = consts.tile([128, NKT], F32)
        epst = consts.tile([128, 1], F32)
        nc.vector.memset(epst, EPS)

        # late-phase weights: tiles allocated here (pool stack order), DMAs
        # emitted inside phase 1 after the critical slice-0 loads
        wlate = ctx.enter_context(tc.tile_pool(name="wlate", bufs=1))
        wo_sb = wlate.tile([64, H, D], BF16)
        w1_sb = wlate.tile([128, NDC, DFF], BF16)
        w2_sb = wlate.tile([128, NFC, D], BF16)
        b1_sb = wlate.tile([128, NFC], F32)
        b2b = g1b = be1b = g2b = be2b = None
        if apply_b2:
            b2b = wlate.tile([128, D], F32)
        if apply_affine1:
            g1b = wlate.tile([128, D], F32)
            be1b = wlate.tile([128, D], F32)
        if apply_affine2:
            g2b = wlate.tile([128, D], F32)
            be2b = wlate.tile([128, D], F32)

        def emit_wlate_dmas():
            nc.sync.dma_start(out=wo_sb, in_=wo.ap().rearrange("(h p) n -> p h n", p=64))
            nc.sync.dma_start(out=w1_sb, in_=w1.ap().rearrange("(c p) n -> p c n", p=128))
            nc.sync.dma_start(out=w2_sb, in_=w2.ap().rearrange("(f p) n -> p f n", p=128))
            nc.sync.dma_start(out=b1_sb, in_=b1.ap().rearrange("(f p) -> p f", p=128))
            if apply_b2:
                nc.sync.dma_start(out=b2b, in_=bcast_ap(b2, 128))
            if apply_affine1:
                nc.sync.dma_start(out=g1b, in_=bcast_ap(g1, 128))
                nc.sync.dma_start(out=be1b, in_=bcast_ap(be1, 128))
            if apply_affine2:
                nc.sync.dma_start(out=g2b, in_=bcast_ap(g2, 128))
                nc.sync.dma_start(out=be2b, in_=bcast_ap(be2, 128))

        # normalized ctx^T per head — written in phase 2, read in phase 3;
        # allocated before the attn pool so pools release in stack order
        ln1 = ctx.enter_context(tc.tile_pool(name="ln1", bufs=1))
        cn = ln1.tile([64, H, SHARD], BF16)

        # ---- residents for phases 1-2 ---------------------------------
        es_attn = contextlib.ExitStack()
        attn_pool = es_attn.enter_context(tc.tile_pool(name="attn", bufs=1))
        kTt = attn_pool.tile([128, NPAIR, S], BF16)          # pair-packed K^T
        # V + ones col, fp8, kt-pair-interleaved for DoubleRow ctx matmuls;
        # +2 pad makes the DoubleRow Ko stride (H*(DK+2) = 528B) 16-aligned
        v1 = attn_pool.tile([128, NKP, 2, H, DK + 2], FP8)
        nc.vector.memset(v1[:, :, :, :, DK:DK + 1], 1.0)
        qT = attn_pool.tile([128, NPAIR, SHARD], BF16)       # pair-packed Q^T

        # ---- phase 1: fp8 DoubleRow projections -----------------------
        SQ8 = float(EXA / (8.0 * np.sqrt(DK)))
        with tc.tile_pool(name="pw", bufs=1) as pw, \
             tc.tile_pool(name="p1p", bufs=3, space="PSUM") as p1p:
            wk_sb = pw.tile([128, NJC, 2, D], FP8)
            nc.sync.dma_start(out=wk_sb, in_=wk8.ap())
            wv_sb = pw.tile([128, NJC, 2, D], FP8)
            nc.sync.dma_start(out=wv_sb, in_=wv8.ap())
            xT_sb = pw.tile([128, NJC, 2, S], FP8)

            def load_slice(sl):
                s0 = sl * 512
                nc.sync.dma_start(out=xT_sb[:, :, :, s0:s0 + 512],
                                  in_=xT8.ap()[:, :, :, s0:s0 + 512])

            for sl in range(2):
                load_slice(sl)
            wq_sb = pw.tile([128, NJC, 2, D], FP8)
            nc.sync.dma_start(out=wq_sb, in_=wq8.ap())
            xqTs = pw.tile([128, NJC, 2, SHARD], FP8)
            nc.sync.dma_start(out=xqTs, in_=xqT8.ap())
            nc.sync.dma_start(out=mbt_t, in_=mbt.ap())
            nc.sync.dma_start(out=mbr_t, in_=mbr.ap())
            for sl in range(2, NSLICE):
                load_slice(sl)
            emit_wlate_dmas()

            def emit_proj(sl):
                """K^T and V matmuls for full-batch slice sl (fp8 DR)."""
                s0 = sl * 512
                for pr in range(NPAIR):
                    kp = p1p.tile([128, 512], F32, tag="kpsum")
                    for j in range(NJC):
                        nc.tensor.matmul(kp, wk_sb[:, j, :, pr * 128:(pr + 1) * 128],
                                         xT_sb[:, j, :, s0:s0 + 512],
                                         start=(j == 0), stop=(j == NJC - 1),
                                         perf_mode=PM_DR)
                    nc.scalar.activation(out=kTt[:, pr, s0:s0 + 512], in_=kp,
                                         func=AF.Copy, bias=0.0, scale=0.125)
                for m in range(4):
                    vp = p1p.tile([128, 512], F32, tag="kpsum")
                    for j in range(NJC):
                        nc.tensor.matmul(vp, xT_sb[:, j, :, s0 + m * 128:s0 + (m + 1) * 128],
                                         wv_sb[:, j, :, :],
                                         start=(j == 0), stop=(j == NJC - 1),
                                         perf_mode=PM_DR)
                    kt_ = sl * 4 + m
                    # vp holds 8*v; the 1/8 is folded into wo host-side
                    nc.vector.tensor_copy(
                        out=v1[:, kt_ // 2, kt_ % 2, :, 0:DK], in_=vp)

            def emit_q(qsl):
                for pr in range(NPAIR):
                    qp = p1p.tile([128, 512], F32, tag="kpsum")
                    for j in range(NJC):
                        nc.tensor.matmul(qp, wq_sb[:, j, :, pr * 128:(pr + 1) * 128],
                                         xqTs[:, j, :, qsl * 512:(qsl + 1) * 512],
                                         start=(j == 0), stop=(j == NJC - 1),
                                         perf_mode=PM_DR)
                    nc.scalar.activation(out=qT[:, pr, qsl * 512:(qsl + 1) * 512],
                                         in_=qp, func=AF.Copy, bias=0.0, scale=SQ8)

            for sl in range(NSLICE):
                emit_proj(sl)
            for qsl in range(NQSL):
                emit_q(qsl)

        # ---- phase 2: attention ---------------------------------------
        with tc.tile_pool(name="ppool", bufs=6) as ppool, \
             tc.tile_pool(name="rpool", bufs=3) as rpool, \
             tc.tile_pool(name="spsum", bufs=2, space="PSUM") as spsum, \
             tc.tile_pool(name="cpsum", bufs=2, space="PSUM") as cpsum:
            for pr in range(NPAIR):
                cA = cpsum.tile([DK + 1, SHARD], F32, tag="ctx")
                cB = cpsum.tile([DK + 1, SHARD], F32, tag="ctx")
                # scores/exp stream per kt; fp8 DoubleRow ctx per kt-PAIR,
                # emitted 1.5 pairs behind so the ~1.2us exp never stalls
                # the PE.  pT tiles hold both kts of a pair interleaved.
                cur = {}
                hist = {}
                for kt in range(NKT + 2):
                    if kt < NKT:
                        j = kt % 2
                        for qh in range(NQSL):
                            sp = spsum.tile([128, SHARD], F32, tag="scores")
                            for hh in (0, 1):
                                lo, hi = hh * 64, hh * 64 + 64
                                nc.tensor.matmul(
                                    sp[:, hh * 512:(hh + 1) * 512],
                                    kTt[lo:hi, pr, kt * 128:(kt + 1) * 128],
                                    qT[lo:hi, pr, qh * 512:(qh + 1) * 512],
                                    start=True, stop=True)
                            if j == 0:
                                pTnew = ppool.tile([128, 2, SHARD], I8,
                                                   tag=f"pT{qh}")
                                cur[qh] = pTnew
                            pT = cur[qh]
                            # scores arrive pre-scaled by EXA (folded into
                            # wq host-side). Strict per-kt alternation keeps
                            # both engines under the PE's per-kt budget.
                            if qh == (kt % 2):
                                # ACT path: true exp (with the 2^K8 shift
                                # folded into the bias), fp8 output
                                nc.scalar.activation(
                                    pT[:, j, :].bitcast(FP8), sp, AF.Exp,
                                    bias=mbr_t[:, kt:kt + 1], scale=1.0 / EXA)
                            else:
                                # DVE path: Schraudolph bit trick with a
                                # fused clamp: int8(max(s' + bias, 0))
                                nc.vector.tensor_scalar(
                                    out=pT[:, j, :], in0=sp,
                                    scalar1=mbt_t[:, kt:kt + 1], scalar2=0.0,
                                    op0=ALU.add, op1=ALU.max)
                        if j == 1:
                            hist[kt // 2] = (cur[0], cur[1])
                    if kt % 2 == 1 and kt >= 3:
                        ktp = (kt - 3) // 2
                        p0, p1 = hist.pop(ktp)
                        p08, p18 = p0.bitcast(FP8), p1.bitcast(FP8)
                        for hh, cps in ((0, cA), (1, cB)):
                            h = 2 * pr + hh
                            va = v1[:, ktp, 0, h, 0:DK + 1]
                            lhsT = bass.AP(
                                tensor=va.tensor, offset=va.offset,
                                ap=[list(va.ap[0]), [H * (DK + 2), 2],
                                    [1, DK + 1]])
                            for qh, p8 in ((0, p08), (1, p18)):
                                nc.tensor.matmul(
                                    cps[:, qh * 512:(qh + 1) * 512],
                                    lhsT,
                                    p8[:, :, hh * 512:(hh + 1) * 512],
                                    start=(ktp == 0), stop=(ktp == NKP - 1),
                                    perf_mode=PM_DR)
                # normalize: stage ctx to SBUF first (frees the psum bank
                # immediately -- the next pair's ctx accumulators and
                # phase 3's wo accumulators reuse those banks), approx-
                # recip the 1-partition rowsum, gpsimd-broadcast the
                # reciprocal, then multiply: pairs 0-2 on gpsimd, last
                # pair on DVE (whose exp work is done by then).
                last = (pr == NPAIR - 1)
                for hh, cps in ((0, cA), (1, cB)):
                    h = 2 * pr + hh
                    # NOTE: the rowsum lives at psum partition 64; only the
                    # ACT copy can shift it to partition 0 (DVE/gpsimd lanes
                    # are partition-hardwired -- a DVE op with in/out at
                    # different base partitions silently breaks on HW).
                    rsr = rpool.tile([1, SHARD], F32, tag="rsr")
                    nc.scalar.copy(out=rsr, in_=cps[DK:DK + 1, :])
                    stg = rpool.tile([DK, SHARD], F32, tag="stg")
                    if last:
                        # DVE is exp-idle by now; evacuating on DVE runs in
                        # parallel with the ACT rsr copies, freeing the ctx
                        # psum banks ~2us sooner for phase 3's wo pools
                        nc.vector.tensor_copy(out=stg, in_=cps[0:DK, :])
                    else:
                        nc.scalar.copy(out=stg, in_=cps[0:DK, :])
                    rb = rpool.tile([64, SHARD], F32, tag="rb")
                    nc.gpsimd.partition_broadcast(rb, rsr)
                    nc.vector.reciprocal_approx_fast(out=rb, in_=rb)
                    if last:
                        nc.vector.tensor_mul(out=cn[:, h, :],
                                             in0=stg[0:DK, :], in1=rb)
                    else:
                        nc.gpsimd.tensor_mul(out=cn[:, h, :],
                                             in0=stg[0:DK, :], in1=rb)
        es_attn.close()  # free kTt + v1 + qT

        # ---- phases 3-5: wo+LN1, x1^T, FFN1, FFN2+LN2 -----------------
        # wo runs in two waves over 8 psum banks: heads 0-5 for every row
        # tile first (only needs pairs 0-2), then heads 6-7 + the LN chain.
        # The first wave (~11us of PE work) hides the last pair's softmax
        # normalization latency, so the PE never idles at the boundary and
        # HAM stays warm into the FFN.
        late2 = ctx.enter_context(tc.tile_pool(name="late2", bufs=1))
        x1T = late2.tile([128, NDC, SHARD], BF16)
        x1keep = late2.tile([128, NQT, D], F32)
        h1T = late2.tile([128, NFC, SHARD], BF16)
        with tc.tile_pool(name="p3s", bufs=1) as p3s, \
             tc.tile_pool(name="p3t", bufs=4) as p3t, \
             tc.tile_pool(name="p3st", bufs=4) as p3st, \
             tc.tile_pool(name="p5s", bufs=3) as p5s, \
             tc.tile_pool(name="p5st", bufs=4) as p5st, \
             contextlib.ExitStack() as es_ffn:
            # two 4-bank pools: pool A reuses the scores psum banks (freed
            # at the last exp) so wo can start before the ctx banks drain
            es_ap = contextlib.ExitStack()
            p3apA = es_ap.enter_context(
                tc.tile_pool(name="p3apA", bufs=1, space="PSUM"))
            p3apB = es_ap.enter_context(
                tc.tile_pool(name="p3apB", bufs=1, space="PSUM"))
            aps = {}
            xts = {}

            def emit_wo_a(m):
                pool = p3apA if m < 4 else p3apB
                ap_ = pool.tile([128, D], F32, tag=f"ap{m}")
                xt = p3s.tile([128, D], F32, tag=f"xres{m}")
                nc.sync.dma_start(out=xt, in_=xq[m * 128:(m + 1) * 128, :])
                aps[m], xts[m] = ap_, xt
                for h in range(6):
                    nc.tensor.matmul(ap_, cn[:, h, m * 128:(m + 1) * 128],
                                     wo_sb[:, h, :], start=(h == 0), stop=False)

            def emit_wo_b(m):
                ap_, xt = aps[m], xts[m]
                for h in (6, 7):
                    nc.tensor.matmul(ap_, cn[:, h, m * 128:(m + 1) * 128],
                                     wo_sb[:, h, :], start=False,
                                     stop=(h == H - 1))
                t = p3t.tile([128, D], F32, tag="tres")
                nc.vector.tensor_add(out=t, in0=ap_, in1=xt)
                stats = p3st.tile([128, 6], F32, tag="stats")
                nc.vector.bn_stats(out=stats, in_=t)
                mv = p3st.tile([128, 2], F32, tag="mv")
                nc.vector.bn_aggr(out=mv, in_=stats)
                sd = p3st.tile([128, 1], F32, tag="sd")
                nc.scalar.activation(out=sd, in_=mv[:, 1:2], func=AF.Sqrt,
                                     bias=epst, scale=1.0)
                rs = p3st.tile([128, 1], F32, tag="rs")
                nc.vector.reciprocal_approx_fast(out=rs, in_=sd)
                x1m = x1keep[:, m, :]
                nc.vector.tensor_scalar(out=x1m, in0=t, scalar1=mv[:, 0:1],
                                        scalar2=rs, op0=ALU.subtract, op1=ALU.mult)
                if apply_affine1:
                    nc.vector.tensor_mul(out=x1m, in0=x1m, in1=g1b)
                    nc.vector.tensor_add(out=x1m, in0=x1m, in1=be1b)

            def emit_x1t(m):
                # transpose into the psum bank freed by this m's wo
                # accumulator (same pool tag -> same bank, dep-ordered)
                x1m = x1keep[:, m, :]
                pool = p3apA if m < 4 else p3apB
                tp = pool.tile([128, 512], F32, tag=f"ap{m}")
                for c in range(NDC):
                    nc.tensor.transpose(tp[:, c * 128:(c + 1) * 128],
                                        x1m[:, c * 128:(c + 1) * 128], ident)
                nc.scalar.copy(out=x1T[:, :, m * 128:(m + 1) * 128], in_=tp)

            def emit_ffn1(qh, p4p):
                for f in range(NFC):
                    hp = p4p.tile([128, 512], F32, tag="hpsum")
                    for c in range(NDC):
                        nc.tensor.matmul(hp,
                                         w1_sb[:, c, f * 128:(f + 1) * 128],
                                         x1T[:, c, qh * 512:(qh + 1) * 512],
                                         start=(c == 0), stop=(c == NDC - 1))
                    nc.scalar.activation(out=h1T[:, f, qh * 512:(qh + 1) * 512],
                                         in_=hp, func=AF.Relu,
                                         bias=b1_sb[:, f:f + 1], scale=1.0)

            def emit_ffn2(m, p5p):
                fp = p5p.tile([128, D], F32, tag="fpsum")
                for f in range(NFC):
                    nc.tensor.matmul(fp, h1T[:, f, m * 128:(m + 1) * 128],
                                     w2_sb[:, f, :], start=(f == 0),
                                     stop=(f == NFC - 1))
                t2 = p5s.tile([128, D], F32, tag="t2")
                nc.vector.tensor_add(out=t2, in0=fp, in1=x1keep[:, m, :])
                if apply_b2:
                    nc.vector.tensor_add(out=t2, in0=t2, in1=b2b)
                stats = p5st.tile([128, 6], F32, tag="stats5")
                nc.vector.bn_stats(out=stats, in_=t2)
                mv = p5st.tile([128, 2], F32, tag="mv5")
                nc.vector.bn_aggr(out=mv, in_=stats)
                sd = p5st.tile([128, 1], F32, tag="sd5")
                nc.scalar.activation(out=sd, in_=mv[:, 1:2], func=AF.Sqrt,
                                     bias=epst, scale=1.0)
                rs = p5st.tile([128, 1], F32, tag="rs5")
                nc.vector.reciprocal_approx_fast(out=rs, in_=sd)
                o = p5s.tile([128, D], F32, tag="otile")
                nc.vector.tensor_scalar(out=o, in0=t2, scalar1=mv[:, 0:1],
                                        scalar2=rs, op0=ALU.subtract, op1=ALU.mult)
                if apply_affine2:
                    nc.vector.tensor_mul(out=o, in0=o, in1=g2b)
                    nc.vector.tensor_add(out=o, in0=o, in1=be2b)
                nc.sync.dma_start(out=out[m * 128:(m + 1) * 128, :], in_=o)

            for m in range(NQT):
                emit_wo_a(m)
            # interleave the h6/h7 tails with the x1 transposes: tp(m)
            # waits on LN(m) (DVE), which trails wo_b(m) by ~2.5us, so
            # keep 3 wo_b's of PE work in between.
            for m in range(3):
                emit_wo_b(m)
            for m in range(NQT):
                if m + 3 < NQT:
                    emit_wo_b(m + 3)
                emit_x1t(m)
            es_ap.close()  # free the 8 wo psum banks
            p4p = es_ffn.enter_context(
                tc.tile_pool(name="p4p", bufs=3, space="PSUM"))
            emit_ffn1(0, p4p)
            p5p = es_ffn.enter_context(
                tc.tile_pool(name="p5p", bufs=2, space="PSUM"))
            for m in range(4):
                emit_ffn2(m, p5p)
            emit_ffn1(1, p4p)
            for m in range(4, NQT):
                emit_ffn2(m, p5p)

    nc.compile()
    return nc


_PROG_CACHE = {}


def _get_program(key):
    if key not in _PROG_CACHE:
        _PROG_CACHE[key] = _build_program(*key)
    return _PROG_CACHE[key]


def _make_in_maps(x, mask, wq, wk, wv, wo, w1, b1, w2, b2, g1, be1, g2, be2):
    f = np.float32
    bf = ml_dtypes.bfloat16
    f8 = ml_dtypes.float8_e4m3fn
    # q/k/v weights x8 into fp8 (sigma ~0.35, comfortably normal-range);
    # the attention scale + fast-exp EXA ride the qT copy scale on-device,
    # and v arrives x8 so wo absorbs a 1/8.
    mraw0 = np.where(np.asarray(mask)[:, 0, 0, :] == 0, f(-1e9), f(0.0))
    mtrick = mraw0 * f(EXA) + f(EXB)
    mraw = mraw0 + f(K8 * np.log(2.0))  # ACT path: fold the 2^K8 shift

    def wlay(w):    # [D, D] -> DR layout [128, NJC, 2, D]
        a = (np.asarray(w, f) * 8).astype(f8)
        return np.ascontiguousarray(a.reshape(NJC, 2, 128, D).transpose(2, 0, 1, 3))

    shared = dict(
        wq8=wlay(wq), wk8=wlay(wk), wv8=wlay(wv),
        wo=(np.asarray(wo, f) / 8).astype(bf), w1=np.asarray(w1, f).astype(bf),
        b1=np.ascontiguousarray(b1, f), w2=np.asarray(w2, f).astype(bf),
        b2=np.ascontiguousarray(b2, f), g1=np.ascontiguousarray(g1, f),
        be1=np.ascontiguousarray(be1, f), g2=np.ascontiguousarray(g2, f),
        be2=np.ascontiguousarray(be2, f),
    )
    # pre-transposed fp8 activations per batch in the DR layout
    xt_list = []
    for b in range(B):
        x8 = np.asarray(x[b], f).T.astype(f8)          # [D, S]
        xt_list.append(np.ascontiguousarray(
            x8.reshape(NJC, 2, 128, S).transpose(2, 0, 1, 3)))
    in_maps = []
    for c in range(N_CORES):
        b, sh = c // 4, c % 4
        m = dict(shared)
        m["xT8"] = xt_list[b]
        m["xqT8"] = np.ascontiguousarray(
            xt_list[b][:, :, :, sh * SHARD:(sh + 1) * SHARD])
        m["xq"] = np.ascontiguousarray(x[b, sh * SHARD:(sh + 1) * SHARD], f)
        m["mbr"] = np.ascontiguousarray(mraw[b].reshape(NKT, 128).T)
        m["mbt"] = np.ascontiguousarray(mtrick[b].reshape(NKT, 128).T)
        in_maps.append(m)
    return in_maps


def kernel(x, mask, wq, wk, wv, wo, w1, b1, w2, b2, g1, be1, g2, be2,
           _trace=False, _tmpdir=None):
    key = (
        not (np.all(np.asarray(g1) == 1.0) and np.all(np.asarray(be1) == 0.0)),
        not (np.all(np.asarray(g2) == 1.0) and np.all(np.asarray(be2) == 0.0)),
        not np.all(np.asarray(b2) == 0.0),
    )
    nc = _get_program(key)
    in_maps = _make_in_maps(x, mask, wq, wk, wv, wo, w1, b1, w2, b2,
                            g1, be1, g2, be2)
    res = None
    for attempt in range(3):
        try:
            res = run_bass_kernel_spmd(nc, in_maps, list(range(N_CORES)),
                                       trace=_trace, tmpdir=_tmpdir)
            break
        except Exception:
            if attempt == 2:
                raise
            import time as _time
            _time.sleep(2.0)
    outs = [res.results[c]["out"] for c in range(N_CORES)]
    full = np.empty((B, S, D), np.float32)
    for c in range(N_CORES):
        b, sh = c // 4, c % 4
        full[b, sh * SHARD:(sh + 1) * SHARD] = outs[c]
    kernel._last_results = res
    return full



# revision 44
# speedup vs baseline: 1.3014x; 1.0942x over previous
"""Self-contained Trainium2 Bass kernel for a Transformer encoder layer.

Reference computation (fp32):
    q,k,v = x@wq, x@wk, x@wv          (per-head split, DK=64)
    attn  = softmax(q@k^T/sqrt(DK) + mask_bias) @ v
    x1    = LN(x + attn@wo) * g1 + be1
    out   = LN(x1 + relu(x1@w1 + b1)@w2 + b2) * g2 + be2

Sharding: pure data-parallel over (batch, seq). 8 cores; core c owns batch
c//4 and a 1024-row query shard (c%4). K/V projections for the full batch
are computed redundantly on each core (no collectives).

Key implementation choices (v3):
  - x arrives host-pre-transposed in the fp8 DoubleRow layout, so phase 1
    is pure DR matmul streaming (no PE transposes, no DVE staging copies):
    q/k/v projections run as fp8e4m3 DoubleRow MMs (weights host-scaled x8
    for e4m3 range; corrective scales ride the psum->sbuf copy affine).
  - FFN / wo matmul operands bf16 (host-cast); psum accumulation fp32.
  - K^T / V / Q^T stay SBUF-resident (no DRAM spill, no phase barrier).
  - phase 3 opens all 8 wo psum accumulators across two 4-bank pools
    (pool A reuses the early-freed scores banks), emits wo heads 0-5 for
    every row tile before heads 6-7, so ~11us of PE work covers the last
    attention pair's softmax normalization and HAM never re-throttles at
    the phase boundary; x1^T transposes then reuse each tile's freed bank.
  - softmax exp runs mostly on the Vector engine via the Schraudolph bit
    trick (int16(s*A+B) bitcast bf16, ~+-3% relative, largely cancelling
    between numerator and rowsum), a fraction on ACT (true Exp) for load
    balance. Mask bias folds into the trick's additive constant; fully
    masked scores saturate the int16 convert to -32768 -> bf16 -0.0.
  - softmax rowsum rides the ctx matmul as a ones column of V (M=65);
    normalization = ACT row copy + gpsimd partition_broadcast +
    reciprocal_approx_fast + one DVE multiply, all on-chip.
  - 1-deep software pipelines (scores(kt+1) ahead of ctx(kt), transposes
    (sl+1) ahead of proj MMs(sl)) keep the PE from stalling on exp/copies.
"""

import os
import sys

import numpy as np

if os.path.isdir("/opt/trn_rl_repo") and "/opt/trn_rl_repo" not in sys.path:
    sys.path.insert(0, "/opt/trn_rl_repo")

import ml_dtypes

import concourse.bacc as bacc
import concourse.bass as bass
import concourse.tile as tile
from concourse import mybir
from concourse.bass_utils import run_bass_kernel_spmd
from concourse.masks import make_identity

B, S, D, H, DK = 2, 4096, 512, 8, 64
DFF = 2048
EPS = 1e-5
N_CORES = 8
SHARD = S // 4  # 1024 query rows per core
F32 = mybir.dt.float32
BF16 = mybir.dt.bfloat16
I16 = mybir.dt.int16
I8 = mybir.dt.int8
FP8 = mybir.dt.float8e4
AF = mybir.ActivationFunctionType
ALU = mybir.AluOpType
PM_DR = mybir.MatmulPerfMode.DoubleRow

NDC_H = D // 128         # host-side chunk count (= NDC)
NJC = D // 256           # ko-pair chunks for fp8 DoubleRow projections
NSLICE = S // 512        # 8 column slices of x^T
NQSL = SHARD // 512      # 2 slices for the Q shard
NPAIR = H // 2           # 4 head pairs
NKT = S // 128           # 32 key tiles
NQT = SHARD // 128       # 8 query tiles in the shard
NDC = D // 128           # 4 contraction chunks of D
NFC = DFF // 128         # 16 chunks of DFF

# Schraudolph fast-exp constants, fp8e4m3 output via int8 bit trick:
# p~ = bitcast_fp8(int8(max(s*EXA + bias, 0))).  K8 folds a 2^-4 scale
# into every p (softmax is scale-invariant; the rowsum rides along): the
# ACT true-exp path then peaks at e^8.03*2^-4 = 193, under the ~240
# ceiling where the hardware ACT->fp8 convert overflows to inf, and the
# DVE int8 trick peaks at t=117, well under the 127=NaN encoding
# regardless of convert rounding mode (no +0.5: the int8 convert was
# observed to round, unlike the truncating int16 convert).
LOG2E = 1.4426950408889634
EXC = 0.0303
K8 = -4.0
EXA = 8.0 * LOG2E
EXB = (7.0 - EXC) * 8.0 + 8.0 * K8
NKP = NKT // 2           # key-tile pairs (fp8 DoubleRow contracts 2 tiles)


def _build_program(apply_affine1, apply_affine2, apply_b2):
    nc = bacc.Bacc("TRN2", target_bir_lowering=False, debug=False,
                   num_devices=N_CORES)

    # host-pre-transposed activations in the fp8 DoubleRow layout
    # [p, j, k, s] = x[s, 256j+128k+p] -- serves as DR lhsT (V) and DR rhs
    # (K/Q): both sides carry the ko dim ahead of the streamed dim.
    xT8 = nc.declare_dram_parameter("xT8", [128, NJC, 2, S], FP8, isOutput=False)
    xqT8 = nc.declare_dram_parameter("xqT8", [128, NJC, 2, SHARD], FP8, isOutput=False)
    xq = nc.declare_dram_parameter("xq", [SHARD, D], F32, isOutput=False)
    # mask biases, host-packed [p, t] = bias[t*128+p] so the DMA is contiguous
    mbt = nc.declare_dram_parameter("mbt", [128, NKT], F32, isOutput=False)
    mbr = nc.declare_dram_parameter("mbr", [128, NKT], F32, isOutput=False)
    # q/k/v weights: fp8, host-scaled x8 (good e4m3 range); corrective
    # scales ride the free psum->sbuf copy affine (kTt: /8; qT: EXA/(8*8))
    wq8 = nc.declare_dram_parameter("wq8", [128, NJC, 2, D], FP8, isOutput=False)
    wk8 = nc.declare_dram_parameter("wk8", [128, NJC, 2, D], FP8, isOutput=False)
    wv8 = nc.declare_dram_parameter("wv8", [128, NJC, 2, D], FP8, isOutput=False)
    wo = nc.declare_dram_parameter("wo", [D, D], BF16, isOutput=False)
    w1 = nc.declare_dram_parameter("w1", [D, DFF], BF16, isOutput=False)
    b1 = nc.declare_dram_parameter("b1", [DFF], F32, isOutput=False)
    w2 = nc.declare_dram_parameter("w2", [DFF, D], BF16, isOutput=False)
    b2 = nc.declare_dram_parameter("b2", [D], F32, isOutput=False)
    g1 = nc.declare_dram_parameter("g1", [D], F32, isOutput=False)
    be1 = nc.declare_dram_parameter("be1", [D], F32, isOutput=False)
    g2 = nc.declare_dram_parameter("g2", [D], F32, isOutput=False)
    be2 = nc.declare_dram_parameter("be2", [D], F32, isOutput=False)
    out = nc.declare_dram_parameter("out", [SHARD, D], F32, isOutput=True)

    def bcast_ap(vec, parts):
        a = vec if isinstance(vec, bass.AP) else vec.ap()
        ap_dims = [list(d) for d in a.ap]
        if len(ap_dims) > 1 and ap_dims[0][1] == 1:
            ap_dims = ap_dims[1:]
        return bass.AP(tensor=a.tensor, offset=a.offset,
                       ap=[[0, parts]] + ap_dims)

    import contextlib
    with tile.TileContext(nc, pool_alloc_mode="queue") as tc, \
         contextlib.ExitStack() as ctx:
        consts = ctx.enter_context(tc.tile_pool(name="consts", bufs=1))
        ident = consts.tile([128, 128], F32)
        make_identity(nc, ident)
        mbt_t = consts.tile([128, NKT], F32)
        mbr_t = consts.tile([128, NKT], F32)
        epst = consts.tile([128, 1], F32)
        nc.vector.memset(epst, EPS)

        # late-phase weights: tiles allocated here (pool stack order), DMAs
        # emitted inside phase 1 after the critical slice-0 loads
        wlate = ctx.enter_context(tc.tile_pool(name="wlate", bufs=1))
        wo_sb = wlate.tile([64, H, D], BF16)
        w1_sb = wlate.tile([128, NDC, DFF], BF16)
        w2_sb = wlate.tile([128, NFC, D], BF16)
        b1_sb = wlate.tile([128, NFC], F32)
        b2b = g1b = be1b = g2b = be2b = None
        if apply_b2:
            b2b = wlate.tile([128, D], F32)
        if apply_affine1:
            g1b = wlate.tile([128, D], F32)
            be1b = wlate.tile([128, D], F32)
        if apply_affine2:
            g2b = wlate.tile([128, D], F32)
            be2b = wlate.tile([128, D], F32)

        def emit_wlate_dmas():
            nc.sync.dma_start(out=wo_sb, in_=wo.ap().rearrange("(h p) n -> p h n", p=64))
            nc.sync.dma_start(out=w1_sb, in_=w1.ap().rearrange("(c p) n -> p c n", p=128))
            nc.sync.dma_start(out=w2_sb, in_=w2.ap().rearrange("(f p) n -> p f n", p=128))
            nc.sync.dma_start(out=b1_sb, in_=b1.ap().rearrange("(f p) -> p f", p=128))
            if apply_b2:
                nc.sync.dma_start(out=b2b, in_=bcast_ap(b2, 128))
            if apply_affine1:
                nc.sync.dma_start(out=g1b, in_=bcast_ap(g1, 128))
                nc.sync.dma_start(out=be1b, in_=bcast_ap(be1, 128))
            if apply_affine2:
                nc.sync.dma_start(out=g2b, in_=bcast_ap(g2, 128))
                nc.sync.dma_start(out=be2b, in_=bcast_ap(be2, 128))

        # normalized ctx^T per head — written in phase 2, read in phase 3;
        # allocated before the attn pool so pools release in stack order
        ln1 = ctx.enter_context(tc.tile_pool(name="ln1", bufs=1))
        cn = ln1.tile([64, H, SHARD], BF16)

        # ---- residents for phases 1-2 ---------------------------------
        es_attn = contextlib.ExitStack()
        attn_pool = es_attn.enter_context(tc.tile_pool(name="attn", bufs=1))
        kTt = attn_pool.tile([128, NPAIR, S], BF16)          # pair-packed K^T
        # V + ones col, fp8, kt-pair-interleaved for DoubleRow ctx matmuls;
        # +2 pad makes the DoubleRow Ko stride (H*(DK+2) = 528B) 16-aligned
        v1 = attn_pool.tile([128, NKP, 2, H, DK + 2], FP8)
        nc.vector.memset(v1[:, :, :, :, DK:DK + 1], 1.0)
        qT = attn_pool.tile([128, NPAIR, SHARD], BF16)       # pair-packed Q^T

        # ---- phase 1: fp8 DoubleRow projections -----------------------
        SQ8 = float(EXA / (8.0 * np.sqrt(DK)))
        with tc.tile_pool(name="pw", bufs=1) as pw, \
             tc.tile_pool(name="p1p", bufs=3, space="PSUM") as p1p:
            wk_sb = pw.tile([128, NJC, 2, D], FP8)
            nc.sync.dma_start(out=wk_sb, in_=wk8.ap())
            wv_sb = pw.tile([128, NJC, 2, D], FP8)
            nc.sync.dma_start(out=wv_sb, in_=wv8.ap())
            xT_sb = pw.tile([128, NJC, 2, S], FP8)

            def load_slice(sl):
                s0 = sl * 512
                nc.sync.dma_start(out=xT_sb[:, :, :, s0:s0 + 512],
                                  in_=xT8.ap()[:, :, :, s0:s0 + 512])

            for sl in range(2):
                load_slice(sl)
            wq_sb = pw.tile([128, NJC, 2, D], FP8)
            nc.sync.dma_start(out=wq_sb, in_=wq8.ap())
            xqTs = pw.tile([128, NJC, 2, SHARD], FP8)
            nc.sync.dma_start(out=xqTs, in_=xqT8.ap())
            nc.sync.dma_start(out=mbt_t, in_=mbt.ap())
            nc.sync.dma_start(out=mbr_t, in_=mbr.ap())
            for sl in range(2, NSLICE):
                load_slice(sl)
            emit_wlate_dmas()

            def emit_proj(sl):
                """K^T and V matmuls for full-batch slice sl (fp8 DR)."""
                s0 = sl * 512
                for pr in range(NPAIR):
                    kp = p1p.tile([128, 512], F32, tag="kpsum")
                    for j in range(NJC):
                        nc.tensor.matmul(kp, wk_sb[:, j, :, pr * 128:(pr + 1) * 128],
                                         xT_sb[:, j, :, s0:s0 + 512],
                                         start=(j == 0), stop=(j == NJC - 1),
                                         perf_mode=PM_DR)
                    nc.scalar.activation(out=kTt[:, pr, s0:s0 + 512], in_=kp,
                                         func=AF.Copy, bias=0.0, scale=0.125)
                for m in range(4):
                    vp = p1p.tile([128, 512], F32, tag="kpsum")
                    for j in range(NJC):
                        nc.tensor.matmul(vp, xT_sb[:, j, :, s0 + m * 128:s0 + (m + 1) * 128],
                                         wv_sb[:, j, :, :],
                                         start=(j == 0), stop=(j == NJC - 1),
                                         perf_mode=PM_DR)
                    kt_ = sl * 4 + m
                    # vp holds 8*v; the 1/8 is folded into wo host-side
                    nc.vector.tensor_copy(
                        out=v1[:, kt_ // 2, kt_ % 2, :, 0:DK], in_=vp)

            def emit_q(qsl):
                for pr in range(NPAIR):
                    qp = p1p.tile([128, 512], F32, tag="kpsum")
                    for j in range(NJC):
                        nc.tensor.matmul(qp, wq_sb[:, j, :, pr * 128:(pr + 1) * 128],
                                         xqTs[:, j, :, qsl * 512:(qsl + 1) * 512],
                                         start=(j == 0), stop=(j == NJC - 1),
                                         perf_mode=PM_DR)
                    nc.scalar.activation(out=qT[:, pr, qsl * 512:(qsl + 1) * 512],
                                         in_=qp, func=AF.Copy, bias=0.0, scale=SQ8)

            for sl in range(NSLICE):
                emit_proj(sl)
            for qsl in range(NQSL):
                emit_q(qsl)

        # ---- phase 2: attention ---------------------------------------
        # qh is hoisted OUT of the kt loop: each (pair, qh) sweep covers
        # 512 queries, so the two ctx accumulators are one bank each and
        # the scores ring deepens to 3 slots (6 banks).  With exps
        # alternating engines per kt and a slot to spare, ACT and DVE
        # pipeline instead of ping-ponging on a 2-slot ring.
        with tc.tile_pool(name="ppool", bufs=6) as ppool, \
             tc.tile_pool(name="rpool", bufs=3) as rpool, \
             tc.tile_pool(name="spsum", bufs=3, space="PSUM") as spsum, \
             tc.tile_pool(name="cpsum", bufs=2, space="PSUM") as cpsum:
          for pr in range(NPAIR):
            for qh in range(NQSL):
                q0 = qh * 512
                cA = cpsum.tile([DK + 1, 512], F32, tag="ctx")
                cB = cpsum.tile([DK + 1, 512], F32, tag="ctx")
                # scores/exp stream per kt; fp8 DoubleRow ctx per kt-PAIR,
                # emitted 2.5 pairs behind so the ~1.2us exp never stalls
                # the PE.  pT tiles hold both kts of a pair interleaved.
                cur = None
                hist = {}
                for kt in range(NKT + 4):
                    if kt < NKT:
                        j = kt % 2
                        sp = spsum.tile([128, SHARD], F32, tag="scores")
                        for hh in (0, 1):
                            lo, hi = hh * 64, hh * 64 + 64
                            nc.tensor.matmul(
                                sp[:, hh * 512:(hh + 1) * 512],
                                kTt[lo:hi, pr, kt * 128:(kt + 1) * 128],
                                qT[lo:hi, pr, q0:q0 + 512],
                                start=True, stop=True)
                        if j == 0:
                            cur = ppool.tile([128, 2, SHARD], I8, tag="pT")
                        pT = cur
                        # scores arrive pre-scaled by EXA (folded into the
                        # qT copy scale). Per-kt engine alternation keeps
                        # both engines loaded; consecutive kts' exps run
                        # concurrently thanks to the 3-deep ring.
                        if kt % 2 == qh:
                            # ACT path: true exp (with the 2^K8 shift
                            # folded into the bias), fp8 output
                            nc.scalar.activation(
                                pT[:, j, :].bitcast(FP8), sp, AF.Exp,
                                bias=mbr_t[:, kt:kt + 1], scale=1.0 / EXA)
                        else:
                            # DVE path: Schraudolph bit trick with a
                            # fused clamp: int8(max(s' + bias, 0))
                            nc.vector.tensor_scalar(
                                out=pT[:, j, :], in0=sp,
                                scalar1=mbt_t[:, kt:kt + 1], scalar2=0.0,
                                op0=ALU.add, op1=ALU.max)
                        if j == 1:
                            hist[kt // 2] = cur
                    if kt % 2 == 1 and kt >= 5:
                        ktp = (kt - 5) // 2
                        p8 = hist.pop(ktp).bitcast(FP8)
                        for hh, cps in ((0, cA), (1, cB)):
                            h = 2 * pr + hh
                            va = v1[:, ktp, 0, h, 0:DK + 1]
                            lhsT = bass.AP(
                                tensor=va.tensor, offset=va.offset,
                                ap=[list(va.ap[0]), [H * (DK + 2), 2],
                                    [1, DK + 1]])
                            nc.tensor.matmul(
                                cps, lhsT,
                                p8[:, :, hh * 512:(hh + 1) * 512],
                                start=(ktp == 0), stop=(ktp == NKP - 1),
                                perf_mode=PM_DR)
                # normalize this sweep's 512-query half: stage ctx to SBUF
                # first (frees the psum bank immediately -- the next
                # sweep's ctx accumulators and phase 3's wo accumulators
                # reuse those banks), approx-recip the 1-partition rowsum,
                # gpsimd-broadcast the reciprocal, then multiply: early
                # sweeps on gpsimd, last sweep on DVE (exp-idle by then).
                last = (pr == NPAIR - 1 and qh == NQSL - 1)
                for hh, cps in ((0, cA), (1, cB)):
                    h = 2 * pr + hh
                    # NOTE: the rowsum lives at psum partition 64; only the
                    # ACT copy can shift it to partition 0 (DVE/gpsimd lanes
                    # are partition-hardwired -- a DVE op with in/out at
                    # different base partitions silently breaks on HW).
                    rsr = rpool.tile([1, 512], F32, tag="rsr")
                    nc.scalar.copy(out=rsr, in_=cps[DK:DK + 1, :])
                    stg = rpool.tile([DK, 512], F32, tag="stg")
                    if last:
                        # DVE is exp-idle by now; evacuating on DVE runs in
                        # parallel with the ACT rsr copies, freeing the ctx
                        # psum banks ~2us sooner for phase 3's wo pools
                        nc.vector.tensor_copy(out=stg, in_=cps[0:DK, :])
                    else:
                        nc.scalar.copy(out=stg, in_=cps[0:DK, :])
                    rb = rpool.tile([64, 512], F32, tag="rb")
                    nc.gpsimd.partition_broadcast(rb, rsr)
                    nc.vector.reciprocal_approx_fast(out=rb, in_=rb)
                    if last:
                        nc.vector.tensor_mul(out=cn[:, h, q0:q0 + 512],
                                             in0=stg[0:DK, :], in1=rb)
                    else:
                        nc.gpsimd.tensor_mul(out=cn[:, h, q0:q0 + 512],
                                             in0=stg[0:DK, :], in1=rb)
        es_attn.close()  # free kTt + v1 + qT

        # ---- phases 3-5: wo+LN1, x1^T, FFN1, FFN2+LN2 -----------------
        # wo runs in two waves over 8 psum banks: heads 0-5 for every row
        # tile first (only needs pairs 0-2), then heads 6-7 + the LN chain.
        # The first wave (~11us of PE work) hides the last pair's softmax
        # normalization latency, so the PE never idles at the boundary and
        # HAM stays warm into the FFN.
        late2 = ctx.enter_context(tc.tile_pool(name="late2", bufs=1))
        x1T = late2.tile([128, NDC, SHARD], BF16)
        x1keep = late2.tile([128, NQT, D], F32)
        h1T = late2.tile([128, NFC, SHARD], BF16)
        with tc.tile_pool(name="p3s", bufs=1) as p3s, \
             tc.tile_pool(name="p3t", bufs=4) as p3t, \
             tc.tile_pool(name="p3st", bufs=4) as p3st, \
             tc.tile_pool(name="p5s", bufs=3) as p5s, \
             tc.tile_pool(name="p5st", bufs=4) as p5st, \
             contextlib.ExitStack() as es_ffn:
            # two 4-bank pools: pool A reuses the scores psum banks (freed
            # at the last exp) so wo can start before the ctx banks drain
            es_ap = contextlib.ExitStack()
            p3apA = es_ap.enter_context(
                tc.tile_pool(name="p3apA", bufs=1, space="PSUM"))
            p3apB = es_ap.enter_context(
                tc.tile_pool(name="p3apB", bufs=1, space="PSUM"))
            aps = {}
            xts = {}

            def emit_wo_a(m):
                pool = p3apA if m < 4 else p3apB
                ap_ = pool.tile([128, D], F32, tag=f"ap{m}")
                xt = p3s.tile([128, D], F32, tag=f"xres{m}")
                nc.sync.dma_start(out=xt, in_=xq[m * 128:(m + 1) * 128, :])
                aps[m], xts[m] = ap_, xt
                for h in range(6):
                    nc.tensor.matmul(ap_, cn[:, h, m * 128:(m + 1) * 128],
                                     wo_sb[:, h, :], start=(h == 0), stop=False)

            def emit_wo_b(m):
                ap_, xt = aps[m], xts[m]
                for h in (6, 7):
                    nc.tensor.matmul(ap_, cn[:, h, m * 128:(m + 1) * 128],
                                     wo_sb[:, h, :], start=False,
                                     stop=(h == H - 1))
                t = p3t.tile([128, D], F32, tag="tres")
                nc.vector.tensor_add(out=t, in0=ap_, in1=xt)
                stats = p3st.tile([128, 6], F32, tag="stats")
                nc.vector.bn_stats(out=stats, in_=t)
                mv = p3st.tile([128, 2], F32, tag="mv")
                nc.vector.bn_aggr(out=mv, in_=stats)
                sd = p3st.tile([128, 1], F32, tag="sd")
                nc.scalar.activation(out=sd, in_=mv[:, 1:2], func=AF.Sqrt,
                                     bias=epst, scale=1.0)
                rs = p3st.tile([128, 1], F32, tag="rs")
                nc.vector.reciprocal_approx_fast(out=rs, in_=sd)
                x1m = x1keep[:, m, :]
                nc.vector.tensor_scalar(out=x1m, in0=t, scalar1=mv[:, 0:1],
                                        scalar2=rs, op0=ALU.subtract, op1=ALU.mult)
                if apply_affine1:
                    nc.vector.tensor_mul(out=x1m, in0=x1m, in1=g1b)
                    nc.vector.tensor_add(out=x1m, in0=x1m, in1=be1b)

            def emit_x1t(m):
                # transpose into the psum bank freed by this m's wo
                # accumulator (same pool tag -> same bank, dep-ordered)
                x1m = x1keep[:, m, :]
                pool = p3apA if m < 4 else p3apB
                tp = pool.tile([128, 512], F32, tag=f"ap{m}")
                for c in range(NDC):
                    nc.tensor.transpose(tp[:, c * 128:(c + 1) * 128],
                                        x1m[:, c * 128:(c + 1) * 128], ident)
                nc.scalar.copy(out=x1T[:, :, m * 128:(m + 1) * 128], in_=tp)

            def emit_ffn1(qh, p4p):
                for f in range(NFC):
                    hp = p4p.tile([128, 512], F32, tag="hpsum")
                    for c in range(NDC):
                        nc.tensor.matmul(hp,
                                         w1_sb[:, c, f * 128:(f + 1) * 128],
                                         x1T[:, c, qh * 512:(qh + 1) * 512],
                                         start=(c == 0), stop=(c == NDC - 1))
                    nc.scalar.activation(out=h1T[:, f, qh * 512:(qh + 1) * 512],
                                         in_=hp, func=AF.Relu,
                                         bias=b1_sb[:, f:f + 1], scale=1.0)

            def emit_ffn2(m, p5p):
                fp = p5p.tile([128, D], F32, tag="fpsum")
                for f in range(NFC):
                    nc.tensor.matmul(fp, h1T[:, f, m * 128:(m + 1) * 128],
                                     w2_sb[:, f, :], start=(f == 0),
                                     stop=(f == NFC - 1))
                t2 = p5s.tile([128, D], F32, tag="t2")
                nc.vector.tensor_add(out=t2, in0=fp, in1=x1keep[:, m, :])
                if apply_b2:
                    nc.vector.tensor_add(out=t2, in0=t2, in1=b2b)
                stats = p5st.tile([128, 6], F32, tag="stats5")
                nc.vector.bn_stats(out=stats, in_=t2)
                mv = p5st.tile([128, 2], F32, tag="mv5")
                nc.vector.bn_aggr(out=mv, in_=stats)
                sd = p5st.tile([128, 1], F32, tag="sd5")
                nc.scalar.activation(out=sd, in_=mv[:, 1:2], func=AF.Sqrt,
                                     bias=epst, scale=1.0)
                rs = p5st.tile([128, 1], F32, tag="rs5")
                nc.vector.reciprocal_approx_fast(out=rs, in_=sd)
                o = p5s.tile([128, D], F32, tag="otile")
                nc.vector.tensor_scalar(out=o, in0=t2, scalar1=mv[:, 0:1],
                                        scalar2=rs, op0=ALU.subtract, op1=ALU.mult)
                if apply_affine2:
                    nc.vector.tensor_mul(out=o, in0=o, in1=g2b)
                    nc.vector.tensor_add(out=o, in0=o, in1=be2b)
                nc.sync.dma_start(out=out[m * 128:(m + 1) * 128, :], in_=o)

            for m in range(NQT):
                emit_wo_a(m)
            # interleave the h6/h7 tails with the x1 transposes: tp(m)
            # waits on LN(m) (DVE), which trails wo_b(m) by ~2.5us, so
            # keep 3 wo_b's of PE work in between.
            for m in range(3):
                emit_wo_b(m)
            for m in range(NQT):
                if m + 3 < NQT:
                    emit_wo_b(m + 3)
                emit_x1t(m)
            es_ap.close()  # free the 8 wo psum banks
            p4p = es_ffn.enter_context(
                tc.tile_pool(name="p4p", bufs=3, space="PSUM"))
            emit_ffn1(0, p4p)
            p5p = es_ffn.enter_context(
                tc.tile_pool(name="p5p", bufs=2, space="PSUM"))
            for m in range(4):
                emit_ffn2(m, p5p)
            emit_ffn1(1, p4p)
            for m in range(4, NQT):
                emit_ffn2(m, p5p)

    nc.compile()
    return nc


_PROG_CACHE = {}


def _get_program(key):
    if key not in _PROG_CACHE:
        _PROG_CACHE[key] = _build_program(*key)
    return _PROG_CACHE[key]


def _make_in_maps(x, mask, wq, wk, wv, wo, w1, b1, w2, b2, g1, be1, g2, be2):
    f = np.float32
    bf = ml_dtypes.bfloat16
    f8 = ml_dtypes.float8_e4m3fn
    # q/k/v weights x8 into fp8 (sigma ~0.35, comfortably normal-range);
    # the attention scale + fast-exp EXA ride the qT copy scale on-device,
    # and v arrives x8 so wo absorbs a 1/8.
    mraw0 = np.where(np.asarray(mask)[:, 0, 0, :] == 0, f(-1e9), f(0.0))
    mtrick = mraw0 * f(EXA) + f(EXB)
    mraw = mraw0 + f(K8 * np.log(2.0))  # ACT path: fold the 2^K8 shift

    def wlay(w):    # [D, D] -> DR layout [128, NJC, 2, D]
        a = (np.asarray(w, f) * 8).astype(f8)
        return np.ascontiguousarray(a.reshape(NJC, 2, 128, D).transpose(2, 0, 1, 3))

    shared = dict(
        wq8=wlay(wq), wk8=wlay(wk), wv8=wlay(wv),
        wo=(np.asarray(wo, f) / 8).astype(bf), w1=np.asarray(w1, f).astype(bf),
        b1=np.ascontiguousarray(b1, f), w2=np.asarray(w2, f).astype(bf),
        b2=np.ascontiguousarray(b2, f), g1=np.ascontiguousarray(g1, f),
        be1=np.ascontiguousarray(be1, f), g2=np.ascontiguousarray(g2, f),
        be2=np.ascontiguousarray(be2, f),
    )
    # pre-transposed fp8 activations per batch in the DR layout
    xt_list = []
    for b in range(B):
        x8 = np.asarray(x[b], f).T.astype(f8)          # [D, S]
        xt_list.append(np.ascontiguousarray(
            x8.reshape(NJC, 2, 128, S).transpose(2, 0, 1, 3)))
    in_maps = []
    for c in range(N_CORES):
        b, sh = c // 4, c % 4
        m = dict(shared)
        m["xT8"] = xt_list[b]
        m["xqT8"] = np.ascontiguousarray(
            xt_list[b][:, :, :, sh * SHARD:(sh + 1) * SHARD])
        m["xq"] = np.ascontiguousarray(x[b, sh * SHARD:(sh + 1) * SHARD], f)
        m["mbr"] = np.ascontiguousarray(mraw[b].reshape(NKT, 128).T)
        m["mbt"] = np.ascontiguousarray(mtrick[b].reshape(NKT, 128).T)
        in_maps.append(m)
    return in_maps


def kernel(x, mask, wq, wk, wv, wo, w1, b1, w2, b2, g1, be1, g2, be2,
           _trace=False, _tmpdir=None):
    key = (
        not (np.all(np.asarray(g1) == 1.0) and np.all(np.asarray(be1) == 0.0)),
        not (np.all(np.asarray(g2) == 1.0) and np.all(np.asarray(be2) == 0.0)),
        not np.all(np.asarray(b2) == 0.0),
    )
    nc = _get_program(key)
    in_maps = _make_in_maps(x, mask, wq, wk, wv, wo, w1, b1, w2, b2,
                            g1, be1, g2, be2)
    res = None
    for attempt in range(3):
        try:
            res = run_bass_kernel_spmd(nc, in_maps, list(range(N_CORES)),
                                       trace=_trace, tmpdir=_tmpdir)
            break
        except Exception:
            if attempt == 2:
                raise
            import time as _time
            _time.sleep(2.0)
    outs = [res.results[c]["out"] for c in range(N_CORES)]
    full = np.empty((B, S, D), np.float32)
    for c in range(N_CORES):
        b, sh = c // 4, c % 4
        full[b, sh * SHARD:(sh + 1) * SHARD] = outs[c]
    kernel._last_results = res
    return full



# revision 45
# speedup vs baseline: 1.3088x; 1.0056x over previous
"""Self-contained Trainium2 Bass kernel for a Transformer encoder layer.

Reference computation (fp32):
    q,k,v = x@wq, x@wk, x@wv          (per-head split, DK=64)
    attn  = softmax(q@k^T/sqrt(DK) + mask_bias) @ v
    x1    = LN(x + attn@wo) * g1 + be1
    out   = LN(x1 + relu(x1@w1 + b1)@w2 + b2) * g2 + be2

Sharding: pure data-parallel over (batch, seq). 8 cores; core c owns batch
c//4 and a 1024-row query shard (c%4). K/V projections for the full batch
are computed redundantly on each core (no collectives).

Key implementation choices (v3):
  - x arrives host-pre-transposed in the fp8 DoubleRow layout, so phase 1
    is pure DR matmul streaming (no PE transposes, no DVE staging copies):
    q/k/v projections run as fp8e4m3 DoubleRow MMs (weights host-scaled x8
    for e4m3 range; corrective scales ride the psum->sbuf copy affine).
  - FFN / wo matmul operands bf16 (host-cast); psum accumulation fp32.
  - K^T / V / Q^T stay SBUF-resident (no DRAM spill, no phase barrier).
  - phase 3 opens all 8 wo psum accumulators across two 4-bank pools
    (pool A reuses the early-freed scores banks), emits wo heads 0-5 for
    every row tile before heads 6-7, so ~11us of PE work covers the last
    attention pair's softmax normalization and HAM never re-throttles at
    the phase boundary; x1^T transposes then reuse each tile's freed bank.
  - softmax exp runs mostly on the Vector engine via the Schraudolph bit
    trick (int16(s*A+B) bitcast bf16, ~+-3% relative, largely cancelling
    between numerator and rowsum), a fraction on ACT (true Exp) for load
    balance. Mask bias folds into the trick's additive constant; fully
    masked scores saturate the int16 convert to -32768 -> bf16 -0.0.
  - softmax rowsum rides the ctx matmul as a ones column of V (M=65);
    normalization = ACT row copy + gpsimd partition_broadcast +
    reciprocal_approx_fast + one DVE multiply, all on-chip.
  - 1-deep software pipelines (scores(kt+1) ahead of ctx(kt), transposes
    (sl+1) ahead of proj MMs(sl)) keep the PE from stalling on exp/copies.
"""

import os
import sys

import numpy as np

if os.path.isdir("/opt/trn_rl_repo") and "/opt/trn_rl_repo" not in sys.path:
    sys.path.insert(0, "/opt/trn_rl_repo")

import ml_dtypes

import concourse.bacc as bacc
import concourse.bass as bass
import concourse.tile as tile
from concourse import mybir
from concourse.bass_utils import run_bass_kernel_spmd
from concourse.masks import make_identity

B, S, D, H, DK = 2, 4096, 512, 8, 64
DFF = 2048
EPS = 1e-5
N_CORES = 8
SHARD = S // 4  # 1024 query rows per core
F32 = mybir.dt.float32
BF16 = mybir.dt.bfloat16
I16 = mybir.dt.int16
I8 = mybir.dt.int8
FP8 = mybir.dt.float8e4
AF = mybir.ActivationFunctionType
ALU = mybir.AluOpType
PM_DR = mybir.MatmulPerfMode.DoubleRow

NDC_H = D // 128         # host-side chunk count (= NDC)
NJC = D // 256           # ko-pair chunks for fp8 DoubleRow projections
NSLICE = S // 512        # 8 column slices of x^T
NQSL = SHARD // 512      # 2 slices for the Q shard
NPAIR = H // 2           # 4 head pairs
NKT = S // 128           # 32 key tiles
NQT = SHARD // 128       # 8 query tiles in the shard
NDC = D // 128           # 4 contraction chunks of D
NFC = DFF // 128         # 16 chunks of DFF

# Schraudolph fast-exp constants, fp8e4m3 output via int8 bit trick:
# p~ = bitcast_fp8(int8(max(s*EXA + bias, 0))).  K8 folds a 2^-4 scale
# into every p (softmax is scale-invariant; the rowsum rides along): the
# ACT true-exp path then peaks at e^8.03*2^-4 = 193, under the ~240
# ceiling where the hardware ACT->fp8 convert overflows to inf, and the
# DVE int8 trick peaks at t=117, well under the 127=NaN encoding
# regardless of convert rounding mode (no +0.5: the int8 convert was
# observed to round, unlike the truncating int16 convert).
LOG2E = 1.4426950408889634
EXC = 0.0303
K8 = -4.0
EXA = 8.0 * LOG2E
EXB = (7.0 - EXC) * 8.0 + 8.0 * K8
NKP = NKT // 2           # key-tile pairs (fp8 DoubleRow contracts 2 tiles)


def _build_program(apply_affine1, apply_affine2, apply_b2):
    nc = bacc.Bacc("TRN2", target_bir_lowering=False, debug=False,
                   num_devices=N_CORES)

    # host-pre-transposed activations in the fp8 DoubleRow layout
    # [p, j, k, s] = x[s, 256j+128k+p] -- serves as DR lhsT (V) and DR rhs
    # (K/Q): both sides carry the ko dim ahead of the streamed dim.
    xT8 = nc.declare_dram_parameter("xT8", [128, NJC, 2, S], FP8, isOutput=False)
    xqT8 = nc.declare_dram_parameter("xqT8", [128, NJC, 2, SHARD], FP8, isOutput=False)
    xq = nc.declare_dram_parameter("xq", [SHARD, D], F32, isOutput=False)
    # mask biases, host-packed [p, t] = bias[t*128+p] so the DMA is contiguous
    mbt = nc.declare_dram_parameter("mbt", [128, NKT], F32, isOutput=False)
    mbr = nc.declare_dram_parameter("mbr", [128, NKT], F32, isOutput=False)
    # q/k/v weights: fp8, host-scaled x8 (good e4m3 range); corrective
    # scales ride the free psum->sbuf copy affine (kTt: /8; qT: EXA/(8*8))
    wq8 = nc.declare_dram_parameter("wq8", [128, NJC, 2, D], FP8, isOutput=False)
    wk8 = nc.declare_dram_parameter("wk8", [128, NJC, 2, D], FP8, isOutput=False)
    wv8 = nc.declare_dram_parameter("wv8", [128, NJC, 2, D], FP8, isOutput=False)
    wo = nc.declare_dram_parameter("wo", [D, D], BF16, isOutput=False)
    w1 = nc.declare_dram_parameter("w1", [D, DFF], BF16, isOutput=False)
    b1 = nc.declare_dram_parameter("b1", [DFF], F32, isOutput=False)
    w2 = nc.declare_dram_parameter("w2", [DFF, D], BF16, isOutput=False)
    b2 = nc.declare_dram_parameter("b2", [D], F32, isOutput=False)
    g1 = nc.declare_dram_parameter("g1", [D], F32, isOutput=False)
    be1 = nc.declare_dram_parameter("be1", [D], F32, isOutput=False)
    g2 = nc.declare_dram_parameter("g2", [D], F32, isOutput=False)
    be2 = nc.declare_dram_parameter("be2", [D], F32, isOutput=False)
    out = nc.declare_dram_parameter("out", [SHARD, D], F32, isOutput=True)

    def bcast_ap(vec, parts):
        a = vec if isinstance(vec, bass.AP) else vec.ap()
        ap_dims = [list(d) for d in a.ap]
        if len(ap_dims) > 1 and ap_dims[0][1] == 1:
            ap_dims = ap_dims[1:]
        return bass.AP(tensor=a.tensor, offset=a.offset,
                       ap=[[0, parts]] + ap_dims)

    import contextlib
    with tile.TileContext(nc, pool_alloc_mode="queue") as tc, \
         contextlib.ExitStack() as ctx:
        consts = ctx.enter_context(tc.tile_pool(name="consts", bufs=1))
        ident = consts.tile([128, 128], F32)
        make_identity(nc, ident)
        mbt_t = consts.tile([128, NKT], F32)
        mbr_t = consts.tile([128, NKT], F32)
        epst = consts.tile([128, 1], F32)
        nc.vector.memset(epst, EPS)

        # late-phase weights: tiles allocated here (pool stack order), DMAs
        # emitted inside phase 1 after the critical slice-0 loads
        wlate = ctx.enter_context(tc.tile_pool(name="wlate", bufs=1))
        wo_sb = wlate.tile([64, H, D], BF16)
        w1_sb = wlate.tile([128, NDC, DFF], BF16)
        w2_sb = wlate.tile([128, NFC, D], BF16)
        b1_sb = wlate.tile([128, NFC], F32)
        b2b = g1b = be1b = g2b = be2b = None
        if apply_b2:
            b2b = wlate.tile([128, D], F32)
        if apply_affine1:
            g1b = wlate.tile([128, D], F32)
            be1b = wlate.tile([128, D], F32)
        if apply_affine2:
            g2b = wlate.tile([128, D], F32)
            be2b = wlate.tile([128, D], F32)

        def emit_wlate_dmas():
            nc.sync.dma_start(out=wo_sb, in_=wo.ap().rearrange("(h p) n -> p h n", p=64))
            nc.sync.dma_start(out=w1_sb, in_=w1.ap().rearrange("(c p) n -> p c n", p=128))
            nc.sync.dma_start(out=w2_sb, in_=w2.ap().rearrange("(f p) n -> p f n", p=128))
            nc.sync.dma_start(out=b1_sb, in_=b1.ap().rearrange("(f p) -> p f", p=128))
            if apply_b2:
                nc.sync.dma_start(out=b2b, in_=bcast_ap(b2, 128))
            if apply_affine1:
                nc.sync.dma_start(out=g1b, in_=bcast_ap(g1, 128))
                nc.sync.dma_start(out=be1b, in_=bcast_ap(be1, 128))
            if apply_affine2:
                nc.sync.dma_start(out=g2b, in_=bcast_ap(g2, 128))
                nc.sync.dma_start(out=be2b, in_=bcast_ap(be2, 128))

        # normalized ctx^T per head — written in phase 2, read in phase 3;
        # allocated before the attn pool so pools release in stack order
        ln1 = ctx.enter_context(tc.tile_pool(name="ln1", bufs=1))
        cn = ln1.tile([64, H, SHARD], BF16)

        # ---- residents for phases 1-2 ---------------------------------
        es_attn = contextlib.ExitStack()
        attn_pool = es_attn.enter_context(tc.tile_pool(name="attn", bufs=1))
        kTt = attn_pool.tile([128, NPAIR, S], BF16)          # pair-packed K^T
        # V + ones col, fp8, kt-pair-interleaved for DoubleRow ctx matmuls;
        # +2 pad makes the DoubleRow Ko stride (H*(DK+2) = 528B) 16-aligned
        v1 = attn_pool.tile([128, NKP, 2, H, DK + 2], FP8)
        nc.vector.memset(v1[:, :, :, :, DK:DK + 1], 1.0)
        qT = attn_pool.tile([128, NPAIR, SHARD], BF16)       # pair-packed Q^T

        # ---- phase 1: fp8 DoubleRow projections -----------------------
        SQ8 = float(EXA / (8.0 * np.sqrt(DK)))
        with tc.tile_pool(name="pw", bufs=1) as pw, \
             tc.tile_pool(name="p1p", bufs=3, space="PSUM") as p1p:
            wk_sb = pw.tile([128, NJC, 2, D], FP8)
            nc.sync.dma_start(out=wk_sb, in_=wk8.ap())
            wv_sb = pw.tile([128, NJC, 2, D], FP8)
            nc.sync.dma_start(out=wv_sb, in_=wv8.ap())
            xT_sb = pw.tile([128, NJC, 2, S], FP8)

            def load_slice(sl):
                s0 = sl * 512
                nc.sync.dma_start(out=xT_sb[:, :, :, s0:s0 + 512],
                                  in_=xT8.ap()[:, :, :, s0:s0 + 512])

            for sl in range(2):
                load_slice(sl)
            wq_sb = pw.tile([128, NJC, 2, D], FP8)
            nc.sync.dma_start(out=wq_sb, in_=wq8.ap())
            xqTs = pw.tile([128, NJC, 2, SHARD], FP8)
            nc.sync.dma_start(out=xqTs, in_=xqT8.ap())
            nc.sync.dma_start(out=mbt_t, in_=mbt.ap())
            nc.sync.dma_start(out=mbr_t, in_=mbr.ap())
            for sl in range(2, NSLICE):
                load_slice(sl)
            emit_wlate_dmas()

            def emit_proj(sl):
                """K^T and V matmuls for full-batch slice sl (fp8 DR)."""
                s0 = sl * 512
                for pr in range(NPAIR):
                    kp = p1p.tile([128, 512], F32, tag="kpsum")
                    for j in range(NJC):
                        nc.tensor.matmul(kp, wk_sb[:, j, :, pr * 128:(pr + 1) * 128],
                                         xT_sb[:, j, :, s0:s0 + 512],
                                         start=(j == 0), stop=(j == NJC - 1),
                                         perf_mode=PM_DR)
                    nc.scalar.activation(out=kTt[:, pr, s0:s0 + 512], in_=kp,
                                         func=AF.Copy, bias=0.0, scale=0.125)
                for m in range(4):
                    vp = p1p.tile([128, 512], F32, tag="kpsum")
                    for j in range(NJC):
                        nc.tensor.matmul(vp, xT_sb[:, j, :, s0 + m * 128:s0 + (m + 1) * 128],
                                         wv_sb[:, j, :, :],
                                         start=(j == 0), stop=(j == NJC - 1),
                                         perf_mode=PM_DR)
                    kt_ = sl * 4 + m
                    # vp holds 8*v; the 1/8 is folded into wo host-side
                    nc.vector.tensor_copy(
                        out=v1[:, kt_ // 2, kt_ % 2, :, 0:DK], in_=vp)

            def emit_q(qsl):
                for pr in range(NPAIR):
                    qp = p1p.tile([128, 512], F32, tag="kpsum")
                    for j in range(NJC):
                        nc.tensor.matmul(qp, wq_sb[:, j, :, pr * 128:(pr + 1) * 128],
                                         xqTs[:, j, :, qsl * 512:(qsl + 1) * 512],
                                         start=(j == 0), stop=(j == NJC - 1),
                                         perf_mode=PM_DR)
                    nc.scalar.activation(out=qT[:, pr, qsl * 512:(qsl + 1) * 512],
                                         in_=qp, func=AF.Copy, bias=0.0, scale=SQ8)

            for sl in range(NSLICE):
                emit_proj(sl)
            for qsl in range(NQSL):
                emit_q(qsl)

        # ---- phase 2: attention ---------------------------------------
        # qh is hoisted OUT of the kt loop: each (pair, qh) sweep covers
        # 512 queries, so the two ctx accumulators are one bank each and
        # the scores ring deepens to 3 slots (6 banks).  With exps
        # alternating engines per kt and a slot to spare, ACT and DVE
        # pipeline instead of ping-ponging on a 2-slot ring.
        with tc.tile_pool(name="ppool", bufs=6) as ppool, \
             tc.tile_pool(name="rpool", bufs=3) as rpool, \
             tc.tile_pool(name="spsum", bufs=3, space="PSUM") as spsum, \
             tc.tile_pool(name="cpsum", bufs=2, space="PSUM") as cpsum:
          for pr in range(NPAIR):
            for qh in range(NQSL):
                q0 = qh * 512
                cA = cpsum.tile([DK + 1, 512], F32, tag="ctx")
                cB = cpsum.tile([DK + 1, 512], F32, tag="ctx")
                # scores/exp stream per kt; fp8 DoubleRow ctx per kt-PAIR,
                # emitted 2.5 pairs behind so the ~1.2us exp never stalls
                # the PE.  pT tiles hold both kts of a pair interleaved.
                cur = None
                hist = {}
                for kt in range(NKT + 4):
                    if kt < NKT:
                        j = kt % 2
                        sp = spsum.tile([128, SHARD], F32, tag="scores")
                        for hh in (0, 1):
                            lo, hi = hh * 64, hh * 64 + 64
                            nc.tensor.matmul(
                                sp[:, hh * 512:(hh + 1) * 512],
                                kTt[lo:hi, pr, kt * 128:(kt + 1) * 128],
                                qT[lo:hi, pr, q0:q0 + 512],
                                start=True, stop=True)
                        if j == 0:
                            cur = ppool.tile([128, 2, SHARD], I8, tag="pT")
                        pT = cur
                        # scores arrive pre-scaled by EXA (folded into the
                        # qT copy scale). ACT takes 18/32 of the exps (its
                        # op is ~13% cheaper than DVE's, and DVE also owns
                        # the recips), Bresenham-spread so consecutive kts
                        # still mostly alternate and pipeline on the
                        # 3-deep ring.
                        if ((kt + 1) * 18) // 32 > (kt * 18) // 32:
                            # ACT path: true exp (with the 2^K8 shift
                            # folded into the bias), fp8 output
                            nc.scalar.activation(
                                pT[:, j, :].bitcast(FP8), sp, AF.Exp,
                                bias=mbr_t[:, kt:kt + 1], scale=1.0 / EXA)
                        else:
                            # DVE path: Schraudolph bit trick with a
                            # fused clamp: int8(max(s' + bias, 0))
                            nc.vector.tensor_scalar(
                                out=pT[:, j, :], in0=sp,
                                scalar1=mbt_t[:, kt:kt + 1], scalar2=0.0,
                                op0=ALU.add, op1=ALU.max)
                        if j == 1:
                            hist[kt // 2] = cur
                    if kt % 2 == 1 and kt >= 5:
                        ktp = (kt - 5) // 2
                        p8 = hist.pop(ktp).bitcast(FP8)
                        for hh, cps in ((0, cA), (1, cB)):
                            h = 2 * pr + hh
                            va = v1[:, ktp, 0, h, 0:DK + 1]
                            lhsT = bass.AP(
                                tensor=va.tensor, offset=va.offset,
                                ap=[list(va.ap[0]), [H * (DK + 2), 2],
                                    [1, DK + 1]])
                            nc.tensor.matmul(
                                cps, lhsT,
                                p8[:, :, hh * 512:(hh + 1) * 512],
                                start=(ktp == 0), stop=(ktp == NKP - 1),
                                perf_mode=PM_DR)
                # normalize this sweep's 512-query half: stage ctx to SBUF
                # first (frees the psum bank immediately -- the next
                # sweep's ctx accumulators and phase 3's wo accumulators
                # reuse those banks), approx-recip the 1-partition rowsum,
                # gpsimd-broadcast the reciprocal, then multiply: early
                # sweeps on gpsimd, last sweep on DVE (exp-idle by then).
                last = (pr == NPAIR - 1 and qh == NQSL - 1)
                for hh, cps in ((0, cA), (1, cB)):
                    h = 2 * pr + hh
                    # NOTE: the rowsum lives at psum partition 64; only the
                    # ACT copy can shift it to partition 0 (DVE/gpsimd lanes
                    # are partition-hardwired -- a DVE op with in/out at
                    # different base partitions silently breaks on HW).
                    rsr = rpool.tile([1, 512], F32, tag="rsr")
                    nc.scalar.copy(out=rsr, in_=cps[DK:DK + 1, :])
                    stg = rpool.tile([DK, 512], F32, tag="stg")
                    if last:
                        # DVE is exp-idle by now; evacuating on DVE runs in
                        # parallel with the ACT rsr copies, freeing the ctx
                        # psum banks ~2us sooner for phase 3's wo pools
                        nc.vector.tensor_copy(out=stg, in_=cps[0:DK, :])
                    else:
                        nc.scalar.copy(out=stg, in_=cps[0:DK, :])
                    rb = rpool.tile([64, 512], F32, tag="rb")
                    nc.gpsimd.partition_broadcast(rb, rsr)
                    nc.vector.reciprocal_approx_fast(out=rb, in_=rb)
                    if last:
                        nc.vector.tensor_mul(out=cn[:, h, q0:q0 + 512],
                                             in0=stg[0:DK, :], in1=rb)
                    else:
                        nc.gpsimd.tensor_mul(out=cn[:, h, q0:q0 + 512],
                                             in0=stg[0:DK, :], in1=rb)
        es_attn.close()  # free kTt + v1 + qT

        # ---- phases 3-5: wo+LN1, x1^T, FFN1, FFN2+LN2 -----------------
        # wo runs in two waves over 8 psum banks: heads 0-5 for every row
        # tile first (only needs pairs 0-2), then heads 6-7 + the LN chain.
        # The first wave (~11us of PE work) hides the last pair's softmax
        # normalization latency, so the PE never idles at the boundary and
        # HAM stays warm into the FFN.
        late2 = ctx.enter_context(tc.tile_pool(name="late2", bufs=1))
        x1T = late2.tile([128, NDC, SHARD], BF16)
        x1keep = late2.tile([128, NQT, D], F32)
        h1T = late2.tile([128, NFC, SHARD], BF16)
        with tc.tile_pool(name="p3s", bufs=1) as p3s, \
             tc.tile_pool(name="p3t", bufs=4) as p3t, \
             tc.tile_pool(name="p3st", bufs=4) as p3st, \
             tc.tile_pool(name="p5s", bufs=3) as p5s, \
             tc.tile_pool(name="p5st", bufs=4) as p5st, \
             contextlib.ExitStack() as es_ffn:
            # two 4-bank pools: pool A reuses the scores psum banks (freed
            # at the last exp) so wo can start before the ctx banks drain
            es_ap = contextlib.ExitStack()
            p3apA = es_ap.enter_context(
                tc.tile_pool(name="p3apA", bufs=1, space="PSUM"))
            p3apB = es_ap.enter_context(
                tc.tile_pool(name="p3apB", bufs=1, space="PSUM"))
            aps = {}
            xts = {}

            def emit_wo_a(m):
                pool = p3apA if m < 4 else p3apB
                ap_ = pool.tile([128, D], F32, tag=f"ap{m}")
                xt = p3s.tile([128, D], F32, tag=f"xres{m}")
                nc.sync.dma_start(out=xt, in_=xq[m * 128:(m + 1) * 128, :])
                aps[m], xts[m] = ap_, xt
                for h in range(6):
                    nc.tensor.matmul(ap_, cn[:, h, m * 128:(m + 1) * 128],
                                     wo_sb[:, h, :], start=(h == 0), stop=False)

            def emit_wo_b(m):
                ap_, xt = aps[m], xts[m]
                for h in (6, 7):
                    nc.tensor.matmul(ap_, cn[:, h, m * 128:(m + 1) * 128],
                                     wo_sb[:, h, :], start=False,
                                     stop=(h == H - 1))
                t = p3t.tile([128, D], F32, tag="tres")
                nc.vector.tensor_add(out=t, in0=ap_, in1=xt)
                stats = p3st.tile([128, 6], F32, tag="stats")
                nc.vector.bn_stats(out=stats, in_=t)
                mv = p3st.tile([128, 2], F32, tag="mv")
                nc.vector.bn_aggr(out=mv, in_=stats)
                sd = p3st.tile([128, 1], F32, tag="sd")
                nc.scalar.activation(out=sd, in_=mv[:, 1:2], func=AF.Sqrt,
                                     bias=epst, scale=1.0)
                rs = p3st.tile([128, 1], F32, tag="rs")
                nc.vector.reciprocal_approx_fast(out=rs, in_=sd)
                x1m = x1keep[:, m, :]
                nc.vector.tensor_scalar(out=x1m, in0=t, scalar1=mv[:, 0:1],
                                        scalar2=rs, op0=ALU.subtract, op1=ALU.mult)
                if apply_affine1:
                    nc.vector.tensor_mul(out=x1m, in0=x1m, in1=g1b)
                    nc.vector.tensor_add(out=x1m, in0=x1m, in1=be1b)

            def emit_x1t(m):
                # transpose into the psum bank freed by this m's wo
                # accumulator (same pool tag -> same bank, dep-ordered)
                x1m = x1keep[:, m, :]
                pool = p3apA if m < 4 else p3apB
                tp = pool.tile([128, 512], F32, tag=f"ap{m}")
                for c in range(NDC):
                    nc.tensor.transpose(tp[:, c * 128:(c + 1) * 128],
                                        x1m[:, c * 128:(c + 1) * 128], ident)
                nc.scalar.copy(out=x1T[:, :, m * 128:(m + 1) * 128], in_=tp)

            def emit_ffn1(qh, p4p):
                for f in range(NFC):
                    hp = p4p.tile([128, 512], F32, tag="hpsum")
                    for c in range(NDC):
                        nc.tensor.matmul(hp,
                                         w1_sb[:, c, f * 128:(f + 1) * 128],
                                         x1T[:, c, qh * 512:(qh + 1) * 512],
                                         start=(c == 0), stop=(c == NDC - 1))
                    nc.scalar.activation(out=h1T[:, f, qh * 512:(qh + 1) * 512],
                                         in_=hp, func=AF.Relu,
                                         bias=b1_sb[:, f:f + 1], scale=1.0)

            def emit_ffn2(m, p5p):
                fp = p5p.tile([128, D], F32, tag="fpsum")
                for f in range(NFC):
                    nc.tensor.matmul(fp, h1T[:, f, m * 128:(m + 1) * 128],
                                     w2_sb[:, f, :], start=(f == 0),
                                     stop=(f == NFC - 1))
                t2 = p5s.tile([128, D], F32, tag="t2")
                nc.vector.tensor_add(out=t2, in0=fp, in1=x1keep[:, m, :])
                if apply_b2:
                    nc.vector.tensor_add(out=t2, in0=t2, in1=b2b)
                stats = p5st.tile([128, 6], F32, tag="stats5")
                nc.vector.bn_stats(out=stats, in_=t2)
                mv = p5st.tile([128, 2], F32, tag="mv5")
                nc.vector.bn_aggr(out=mv, in_=stats)
                sd = p5st.tile([128, 1], F32, tag="sd5")
                nc.scalar.activation(out=sd, in_=mv[:, 1:2], func=AF.Sqrt,
                                     bias=epst, scale=1.0)
                rs = p5st.tile([128, 1], F32, tag="rs5")
                nc.vector.reciprocal_approx_fast(out=rs, in_=sd)
                o = p5s.tile([128, D], F32, tag="otile")
                nc.vector.tensor_scalar(out=o, in0=t2, scalar1=mv[:, 0:1],
                                        scalar2=rs, op0=ALU.subtract, op1=ALU.mult)
                if apply_affine2:
                    nc.vector.tensor_mul(out=o, in0=o, in1=g2b)
                    nc.vector.tensor_add(out=o, in0=o, in1=be2b)
                nc.sync.dma_start(out=out[m * 128:(m + 1) * 128, :], in_=o)

            for m in range(NQT):
                emit_wo_a(m)
            # interleave the h6/h7 tails with the x1 transposes: tp(m)
            # waits on LN(m) (DVE), which trails wo_b(m) by ~2.5us, so
            # keep 3 wo_b's of PE work in between.
            for m in range(3):
                emit_wo_b(m)
            for m in range(NQT):
                if m + 3 < NQT:
                    emit_wo_b(m + 3)
                emit_x1t(m)
            es_ap.close()  # free the 8 wo psum banks
            p4p = es_ffn.enter_context(
                tc.tile_pool(name="p4p", bufs=3, space="PSUM"))
            emit_ffn1(0, p4p)
            p5p = es_ffn.enter_context(
                tc.tile_pool(name="p5p", bufs=2, space="PSUM"))
            for m in range(4):
                emit_ffn2(m, p5p)
            emit_ffn1(1, p4p)
            for m in range(4, NQT):
                emit_ffn2(m, p5p)

    nc.compile()
    return nc


_PROG_CACHE = {}


def _get_program(key):
    if key not in _PROG_CACHE:
        _PROG_CACHE[key] = _build_program(*key)
    return _PROG_CACHE[key]


def _make_in_maps(x, mask, wq, wk, wv, wo, w1, b1, w2, b2, g1, be1, g2, be2):
    f = np.float32
    bf = ml_dtypes.bfloat16
    f8 = ml_dtypes.float8_e4m3fn
    # q/k/v weights x8 into fp8 (sigma ~0.35, comfortably normal-range);
    # the attention scale + fast-exp EXA ride the qT copy scale on-device,
    # and v arrives x8 so wo absorbs a 1/8.
    mraw0 = np.where(np.asarray(mask)[:, 0, 0, :] == 0, f(-1e9), f(0.0))
    mtrick = mraw0 * f(EXA) + f(EXB)
    mraw = mraw0 + f(K8 * np.log(2.0))  # ACT path: fold the 2^K8 shift

    def wlay(w):    # [D, D] -> DR layout [128, NJC, 2, D]
        a = (np.asarray(w, f) * 8).astype(f8)
        return np.ascontiguousarray(a.reshape(NJC, 2, 128, D).transpose(2, 0, 1, 3))

    shared = dict(
        wq8=wlay(wq), wk8=wlay(wk), wv8=wlay(wv),
        wo=(np.asarray(wo, f) / 8).astype(bf), w1=np.asarray(w1, f).astype(bf),
        b1=np.ascontiguousarray(b1, f), w2=np.asarray(w2, f).astype(bf),
        b2=np.ascontiguousarray(b2, f), g1=np.ascontiguousarray(g1, f),
        be1=np.ascontiguousarray(be1, f), g2=np.ascontiguousarray(g2, f),
        be2=np.ascontiguousarray(be2, f),
    )
    # pre-transposed fp8 activations per batch in the DR layout
    xt_list = []
    for b in range(B):
        x8 = np.asarray(x[b], f).T.astype(f8)          # [D, S]
        xt_list.append(np.ascontiguousarray(
            x8.reshape(NJC, 2, 128, S).transpose(2, 0, 1, 3)))
    in_maps = []
    for c in range(N_CORES):
        b, sh = c // 4, c % 4
        m = dict(shared)
        m["xT8"] = xt_list[b]
        m["xqT8"] = np.ascontiguousarray(
            xt_list[b][:, :, :, sh * SHARD:(sh + 1) * SHARD])
        m["xq"] = np.ascontiguousarray(x[b, sh * SHARD:(sh + 1) * SHARD], f)
        m["mbr"] = np.ascontiguousarray(mraw[b].reshape(NKT, 128).T)
        m["mbt"] = np.ascontiguousarray(mtrick[b].reshape(NKT, 128).T)
        in_maps.append(m)
    return in_maps


def kernel(x, mask, wq, wk, wv, wo, w1, b1, w2, b2, g1, be1, g2, be2,
           _trace=False, _tmpdir=None):
    key = (
        not (np.all(np.asarray(g1) == 1.0) and np.all(np.asarray(be1) == 0.0)),
        not (np.all(np.asarray(g2) == 1.0) and np.all(np.asarray(be2) == 0.0)),
        not np.all(np.asarray(b2) == 0.0),
    )
    nc = _get_program(key)
    in_maps = _make_in_maps(x, mask, wq, wk, wv, wo, w1, b1, w2, b2,
                            g1, be1, g2, be2)
    res = None
    for attempt in range(3):
        try:
            res = run_bass_kernel_spmd(nc, in_maps, list(range(N_CORES)),
                                       trace=_trace, tmpdir=_tmpdir)
            break
        except Exception:
            if attempt == 2:
                raise
            import time as _time
            _time.sleep(2.0)
    outs = [res.results[c]["out"] for c in range(N_CORES)]
    full = np.empty((B, S, D), np.float32)
    for c in range(N_CORES):
        b, sh = c // 4, c % 4
        full[b, sh * SHARD:(sh + 1) * SHARD] = outs[c]
    kernel._last_results = res
    return full

